# revision 1
# baseline (speedup 1.0000x reference)
"""Trainium2 Bass kernel for nn_DecoderLayer_43877385896448 (see spec).

Decoder layer with sigmoid linear attention (rank-1 per head), 2 attn blocks,
FFN, 3 layernorms.  B=4, S=4096, D=1024, H=16 heads (depth-1 q/k per head),
F=4096.

Sharding: rows (b, s) split across 8 cores -> core c owns batch b=c//2,
sequence half (c%2)*2048.  All matmuls are row-parallel with replicated
weights; the only cross-core exchange is an AllReduce of the tiny per-batch
attention state kv[16,65] (one per attention block, overlapped with compute).

Key simplification: attention output = sigmoid(q) @ BD @ wo + bo with
BD = blockdiag(cumsum_h kv), so the [S,D]x[D,D] output projection collapses
to [S,17]x[17,D] via M_aug = [BD @ wo ; bo], eliminating a 34 GFLOP matmul
per block and the [S,D] mha intermediate.

Matmul dtypes: float32r (full-rate fp32, ~1e-4 rounding) for attention,
bf16 for the FFN and the q2 projection (weights cast host-side).
"""

import numpy as np
import ml_dtypes

import concourse.bass as bass
import concourse.bacc as bacc
import concourse.tile as tile
import concourse.mybir as mybir
from concourse import masks
from concourse.bass_utils import run_bass_kernel_spmd

F32 = mybir.dt.float32
F32R = mybir.dt.float32r
BF16 = mybir.dt.bfloat16
AF = mybir.ActivationFunctionType
ALU = mybir.AluOpType
AX = mybir.AxisListType

B, S, D, H, FF = 4, 4096, 1024, 16, 4096
DV = D // H            # 64
P = 128
N_CORES = 8
S_LOC = 2048           # rows per core
T = S_LOC // P         # 16 s-tiles per core
KT = D // P            # 8 k-tiles over D
MT = FF // P           # 32 dff tiles
EPS = 1e-6
import os
NO_CC = bool(int(os.environ.get("BASS_NO_CC", "0")))
SQ = 4                 # ffn processes s in 4 quarters of 512 rows
TQ = T // SQ


def build_program(affine_trivial=False):
    nc = bacc.Bacc("TRN2", target_bir_lowering=False, debug=False,
                   num_devices=N_CORES)

    # ---- DRAM I/O ----
    d = {}

    def din(name, shape, dtype=F32):
        d[name] = nc.dram_tensor(name, list(shape), dtype,
                                 kind="ExternalInput").ap()

    din("x_loc", [S_LOC, D], F32R)
    din("enc_loc", [S_LOC, D])
    for w in ["wq1p", "wk1p", "wk2p"]:
        din(w, [P, KT, H], F32R)
    din("wq2p", [P, KT, H], BF16)
    for w in ["wv1p", "wo1p", "wv2p", "wo2p"]:
        din(w, [P, KT, D], F32R)
    din("wf1p", [MT, P, KT, P], BF16)
    din("wf2p", [P, MT, D], BF16)
    din("bq1c", [H, 1]); din("bq2c", [H, 1])
    din("bk1b", [P, H]); din("bk2b", [P, H])
    din("bv1h", [H, DV]); din("bv2h", [H, DV])
    din("bo1r", [H, D]); din("bo2r", [H, D])
    din("bf1c", [P, MT]); din("bf2b", [P, D], F32R)
    for v in ["g1b", "be1b", "g2b", "be2b", "g3b", "be3b"]:
        din(v, [P, D])
    din("maskh", [H, D]); din("maskT", [P, KT, H])
    din("U16", [H, H]); din("Bsel", [64, H]); din("BselT", [H, 64])
    out_dram = nc.dram_tensor("out_loc", [S_LOC, D], F32,
                              kind="ExternalOutput").ap()

    with tile.TileContext(nc) as tc:
        def pool(name, bufs, side="left", space="SBUF"):
            return tc.alloc_tile_pool(name=name, bufs=bufs, side=side,
                                      space=space)

        # ============ long-lived pools ============
        # LEFT stack (bottom): cpool, row.  RIGHT stack: ln, state, sigq2,
        # sigq1, sigk (popped in reverse order at their phase ends).
        cpool = pool("consts", 1)
        wf1_pool = pool("wf1", 4)
        ln_pool = pool("ln", 2, side="right")
        state_pool = pool("state", 1, side="right")
        sigq2_pool = pool("sigq2", 1, side="right")
        row_pool = pool("rows", 2, side="right")
        sigq1_pool = pool("sigq1", 1, side="right")
        ma1_pool = pool("ma1", 1, side="right")
        sigk_pool = pool("sigk", 1, side="right")

        ident = cpool.tile([P, P], F32, name="ident")
        masks.make_identity(nc, ident[:])
        identr = cpool.tile([P, P], F32R, name="identr")
        nc.vector.tensor_copy(identr[:], ident[:])

        def load_const(key, dtype=F32, pl=None):
            pl = pl if pl is not None else cpool
            t_ = pl.tile([int(s) for s in d[key].shape], dtype,
                         name=f"c_{key}")
            nc.sync.dma_start(t_[:], d[key][:])
            return t_

        maskh = load_const("maskh")
        maskT = load_const("maskT")
        U16 = load_const("U16")
        Bsel = load_const("Bsel")
        BselT = load_const("BselT")
        bq1c = load_const("bq1c"); bq2c = load_const("bq2c")
        bk1b = load_const("bk1b"); bk2b = load_const("bk2b")
        bv1h = load_const("bv1h"); bv2h = load_const("bv2h")
        bo1r = load_const("bo1r"); bo2r = load_const("bo2r")
        bf1c = load_const("bf1c")
        bf2br = load_const("bf2b", F32R)
        wq1 = load_const("wq1p", F32R); wk1 = load_const("wk1p", F32R)
        wq2 = load_const("wq2p", BF16); wk2 = load_const("wk2p", F32R)
        eps = cpool.tile([P, 1], F32, name="epsc")
        nc.vector.memset(eps[:], EPS)
        ones_col = cpool.tile([P, 1], F32, name="ones_col")
        nc.vector.memset(ones_col[:], 1.0)

        dram_pool = pool("ccdram", 1, space="DRAM")
        cc1_in = dram_pool.tile([64, 65], F32, name="cc1_in")
        cc1_out = dram_pool.tile([64, 65], F32, name="cc1_out")
        cc2_in = dram_pool.tile([64, 65], F32, name="cc2_in")
        cc2_out = dram_pool.tile([64, 65], F32, name="cc2_out")
        out1d = dram_pool.tile([S_LOC, D], F32R, name="out1d")
        out2d = dram_pool.tile([S_LOC, D], F32R, name="out2d")

        sigq1 = sigq1_pool.tile([H, S_LOC], F32R, name="sigq1")
        sigq2 = sigq2_pool.tile([H, S_LOC], F32R, name="sigq2")
        sigk1 = sigk_pool.tile([P, T, H], F32R, name="sigk1")
        sigk2 = sigk_pool.tile([P, T, H], F32R, name="sigk2")

        def layernorm(res_ps, out_slice, g_sb, be_sb, lnp, tname):
            """LN of psum tile res_ps [128,1024] -> out_slice (SBUF).
            Stats on DVE, normalize on ScalarE; affine skipped when
            host-side inputs have g==1, be==0 (affine_trivial)."""
            st6 = lnp.tile([P, 2, 6], F32, tag="st6", name=f"st6_{tname}")
            nc.vector.bn_stats(st6[:, 0, :], res_ps[:, 0:512])
            nc.vector.bn_stats(st6[:, 1, :], res_ps[:, 512:1024])
            mv = lnp.tile([P, 2], F32, tag="mv", name=f"mv_{tname}")
            nc.vector.bn_aggr(mv[:], st6[:])
            std = lnp.tile([P, 1], F32, tag="std", name=f"std_{tname}")
            nc.scalar.activation(std[:], mv[:, 1:2], AF.Sqrt, bias=eps[:])
            rstd = lnp.tile([P, 1], F32, tag="rstd", name=f"rstd_{tname}")
            nc.vector.reciprocal(rstd[:], std[:])
            nmr = lnp.tile([P, 1], F32, tag="nmr", name=f"nmr_{tname}")
            nc.vector.scalar_tensor_tensor(nmr[:], mv[:, 0:1], -1.0, rstd[:],
                                           op0=ALU.mult, op1=ALU.mult)
            if affine_trivial:
                nc.scalar.activation(out_slice, res_ps[:], AF.Identity,
                                     bias=nmr[:], scale=rstd[:])
            else:
                xh = lnp.tile([P, 1024], F32, tag="ot", name=f"xh_{tname}")
                nc.scalar.activation(xh[:], res_ps[:], AF.Identity,
                                     bias=nmr[:], scale=rstd[:])
                nc.vector.scalar_tensor_tensor(out_slice, xh[:], 1.0, g_sb[:],
                                               op0=ALU.mult, op1=ALU.mult)
                nc.vector.tensor_add(out_slice, out_slice, be_sb[:])

        def transpose_into(src_row, dst, t, tp_pool, engine, tname):
            """src [128,1024] row tile -> dst[:, kt, t*128:(t+1)*128]."""
            for half in range(2):
                tp = tp_pool.tile([P, 512], F32, tag="tp",
                                  name=f"tp_{tname}_{half}")
                for j in range(4):
                    kt = half * 4 + j
                    nc.tensor.matmul(tp[:, j * P:(j + 1) * P],
                                     src_row[:, kt * P:(kt + 1) * P],
                                     ident[:], is_transpose=True)
                dst_ap = dst[:, half * 4:(half + 1) * 4, t * P:(t + 1) * P]
                src_ap = tp[:].rearrange("p (k n) -> p k n", k=4)
                use_v = (engine == "vector") if engine != "split" \
                    else (half == 0)
                if use_v:
                    nc.vector.tensor_copy(dst_ap, src_ap)
                else:
                    nc.scalar.copy(dst_ap, src_ap)

        def attn_state(G_ps, sel_psum, cc_in, nm):
            """G psum [16,1536] -> kv[16,65] -> batch-slot select -> DMA."""
            gtmp = state_pool.tile([16, 1024], F32, tag="gtmp",
                                   name=f"gtmp_{nm}")
            nc.vector.tensor_mul(gtmp[:], G_ps[:, 0:1024], maskh[:])
            kvp = state_pool.tile([16, 65], F32, tag="kvp", name=f"kvp_{nm}")
            nc.vector.tensor_reduce(
                kvp[:, 0:64], gtmp[:].rearrange("p (c v) -> p v c", v=DV),
                axis=AX.X, op=ALU.add)
            nc.vector.tensor_copy(kvp[:, 64:65], G_ps[:, 1024:1025])
            # kvsel[64,65]: rows [16b:16b+16] = kvp, rest 0 (BselT one-hot)
            kvsel_ps = sel_psum.tile([64, 65], F32, tag="vp",
                                     name=f"kvselp_{nm}")
            nc.tensor.matmul(kvsel_ps[:], BselT[:], kvp[:],
                             start=True, stop=True)
            kvsel = state_pool.tile([64, 65], F32, tag="kvsel",
                                    name=f"kvsel_{nm}")
            nc.vector.tensor_copy(kvsel[:], kvsel_ps[:])
            nc.sync.dma_start(cc_in[:], kvsel[:])

        def state_to_M(cc_out, wo_sb, bvh, bor, spsum, ma_pool, nm):
            """AllReduce out -> own-batch kv -> cumsum -> M_aug [17,1024]."""
            kvred = state_pool.tile([64, 65], F32, tag="kvred",
                                    name=f"kvred_{nm}")
            nc.sync.dma_start(kvred[:], cc_out[:])
            kvmy_ps = spsum.tile([16, 65], F32, tag="sm", name=f"kvmyp_{nm}")
            nc.tensor.matmul(kvmy_ps[:], Bsel[:], kvred[:],
                             start=True, stop=True)
            kvmy = state_pool.tile([16, 65], F32, tag="kvmy",
                                   name=f"kvmy_{nm}")
            nc.vector.tensor_copy(kvmy[:], kvmy_ps[:])
            kv_bv = state_pool.tile([16, 64], F32, tag="kv_bv",
                                    name=f"kv_bv_{nm}")
            nc.vector.scalar_tensor_tensor(
                kv_bv[:], bvh[:], kvmy[:, 64:65], kvmy[:, 0:64],
                op0=ALU.mult, op1=ALU.add)
            scum_ps = spsum.tile([16, 64], F32, tag="sm", name=f"scump_{nm}")
            nc.tensor.matmul(scum_ps[:], U16[:], kv_bv[:],
                             start=True, stop=True)
            scum = state_pool.tile([16, 64], F32, tag="scumsb",
                                   name=f"scum_{nm}")
            nc.vector.tensor_copy(scum[:], scum_ps[:])
            scumT_ps = spsum.tile([64, 16], F32, tag="sm", name=f"scumTp_{nm}")
            nc.tensor.matmul(scumT_ps[:], scum[:], ident[:16, :16],
                             is_transpose=True)
            scumT2 = state_pool.tile([P, 16], F32, tag="scumT2",
                                     name=f"scumT2_{nm}")
            nc.vector.tensor_copy(scumT2[0:64, :], scumT_ps[:])
            nc.vector.tensor_copy(scumT2[64:P, :], scumT_ps[:])
            bdts = []
            for kt in range(KT):
                bdt = state_pool.tile([P, 16], F32R, tag=f"bdt{kt}",
                                      name=f"bdt_{nm}_{kt}")
                nc.vector.tensor_mul(bdt[:], scumT2[:], maskT[:, kt, :])
                bdts.append(bdt)
            Ma = ma_pool.tile([H, 1024], F32R, name=f"Ma_{nm}")
            for n2 in range(2):
                sl = slice(n2 * 512, (n2 + 1) * 512)
                M_ps = spsum.tile([16, 512], F32, tag="sm",
                                  name=f"M_{nm}_{n2}")
                for kt in range(KT):
                    nc.tensor.matmul(M_ps[:], bdts[kt][:], wo_sb[:, kt, sl],
                                     start=(kt == 0), stop=False)
                # += (I/16-colsum=1) @ bo_rep : folds the wo bias into M
                nc.tensor.matmul(M_ps[:], ident[:16, :16], bor[:, sl],
                                 start=False, stop=True)
                nc.vector.tensor_copy(Ma[:, sl], M_ps[:])
            return Ma

        # ================= PHASE A1: x side =================
        ksc_pool = pool("ksc", 2)
        v_pool = pool("v", 1)
        wv1_pool = pool("wv1", 1)
        wv1 = wv1_pool.tile([P, KT, D], F32R, name="wv1")
        for kt in range(KT):
            nc.sync.dma_start(wv1[:, kt, :], d["wv1p"][:, kt, :])
        xT_pool = pool("xT", 1)
        xT = xT_pool.tile([P, KT, S_LOC], F32R, name="xT")
        tpA = pool("tpA", 4, space="PSUM")
        for t in range(T):
            xr = row_pool.tile([P, D], F32R, tag="xr", name=f"xr_{t}")
            nc.sync.dma_start(xr[:], d["x_loc"][t * P:(t + 1) * P, :])
            transpose_into(xr.bitcast(F32), xT, t, tpA, "vector", f"x{t}")
        tpA.release()

        q1p = pool("q1p", 2, space="PSUM")
        for n in range(S_LOC // 512):
            qp = q1p.tile([16, 512], F32, tag="qc", name=f"q1_{n}")
            for kt in range(KT):
                nc.tensor.matmul(qp[:], wq1[:, kt, :],
                                 xT[:, kt, n * 512:(n + 1) * 512],
                                 start=(kt == 0), stop=(kt == KT - 1))
            nc.scalar.activation(sigq1[0:16, n * 512:(n + 1) * 512], qp[:],
                                 AF.Sigmoid, bias=bq1c[:])
        q1p.release()

        k_psum = pool("k_psum", 1, space="PSUM")
        for t in range(T):
            kp = k_psum.tile([P, H], F32, tag="kp", name=f"k1p_{t}")
            for kt in range(KT):
                nc.tensor.matmul(kp[:], xT[:, kt, t * P:(t + 1) * P],
                                 wk1[:, kt, :],
                                 start=(kt == 0), stop=(kt == KT - 1))
            ktmp = ksc_pool.tile([P, H], F32, tag="ktmp", name=f"k1t_{t}")
            nc.vector.tensor_add(ktmp[:], kp[:], bk1b[:])
            nc.scalar.activation(sigk1[:, t, :], ktmp[:], AF.Sigmoid)

        G_psum = pool("G_psum", 1, space="PSUM")
        v_psum = pool("v_psum", 1, space="PSUM")
        G1_ps = G_psum.tile([16, 1536], F32, tag="G", name="G1_ps")
        for t in range(T):
            vp = v_psum.tile([P, D], F32, tag="vp", name=f"v1p_{t}")
            for kt in range(KT):
                for n2 in range(2):
                    nc.tensor.matmul(vp[:, n2 * 512:(n2 + 1) * 512],
                                     xT[:, kt, t * P:(t + 1) * P],
                                     wv1[:, kt, n2 * 512:(n2 + 1) * 512],
                                     start=(kt == 0), stop=(kt == KT - 1))
            vt = v_pool.tile([P, D + 1], F32R, tag="vt", name=f"v1t_{t}")
            nc.vector.tensor_copy(vt[:, 0:D], vp[:])
            nc.vector.tensor_copy(vt[:, D:D + 1], ones_col[:])
            nc.tensor.matmul(G1_ps[:, 0:512], sigk1[:, t, :], vt[:, 0:512],
                             start=(t == 0), stop=(t == T - 1))
            nc.tensor.matmul(G1_ps[:, 512:1024], sigk1[:, t, :],
                             vt[:, 512:1024], start=(t == 0), stop=(t == T - 1))
            nc.tensor.matmul(G1_ps[:, 1024:1025],
                             sigk1[:, t, :].bitcast(F32),
                             vt[:, D:D + 1].bitcast(F32),
                             start=(t == 0), stop=(t == T - 1))
        attn_state(G1_ps, v_psum, cc1_in, "kv1")
        if NO_CC:
            nc.sync.dma_start(cc1_out[:], cc1_in[:])
        else:
            nc.gpsimd.collective_compute(
                "AllReduce", ALU.add, replica_groups=[list(range(N_CORES))],
                ins=[cc1_in.opt()], outs=[cc1_out.opt()])
        xT_pool.release()
        wv1_pool.release()

        # ================= PHASE A2: enc side (overlaps AllReduce 1) ====
        wv2_pool = pool("wv2", 1)
        wv2 = wv2_pool.tile([P, KT, D], F32R, name="wv2")
        for kt in range(KT):
            nc.sync.dma_start(wv2[:, kt, :], d["wv2p"][:, kt, :])
        encT_pool = pool("encT", 1)
        encT = encT_pool.tile([P, KT, S_LOC], F32R, name="encT")
        wo1_pool = pool("wo1", 1)
        wo1 = wo1_pool.tile([P, KT, D], F32R, name="wo1")
        for kt in range(KT):
            nc.sync.dma_start(wo1[:, kt, :], d["wo1p"][:, kt, :])
        tpA2 = pool("tpA2", 2, space="PSUM")
        for t in range(T):
            er = row_pool.tile([P, D], F32, tag="xr", name=f"er_{t}")
            nc.sync.dma_start(er[:], d["enc_loc"][t * P:(t + 1) * P, :])
            transpose_into(er, encT, t, tpA2, "scalar", f"e{t}")
        tpA2.release()

        # m1 state chain: fills PE gaps during the enc-side work; waits AR1
        sps1 = pool("sps1", 1, space="PSUM")
        Ma1 = state_to_M(cc1_out, wo1, bv1h, bo1r, sps1, ma1_pool, "m1")
        sps1.release()

        for t in range(T):
            kp2 = k_psum.tile([P, H], F32, tag="kp", name=f"k2p_{t}")
            for kt in range(KT):
                nc.tensor.matmul(kp2[:], encT[:, kt, t * P:(t + 1) * P],
                                 wk2[:, kt, :],
                                 start=(kt == 0), stop=(kt == KT - 1))
            ktmp2 = ksc_pool.tile([P, H], F32, tag="ktmp", name=f"k2t_{t}")
            nc.vector.tensor_add(ktmp2[:], kp2[:], bk2b[:])
            nc.scalar.activation(sigk2[:, t, :], ktmp2[:], AF.Sigmoid)

        G2_ps = G_psum.tile([16, 1536], F32, tag="G", name="G2_ps")
        for t in range(T):
            vp2 = v_psum.tile([P, D], F32, tag="vp", name=f"v2p_{t}")
            for kt in range(KT):
                for n2 in range(2):
                    nc.tensor.matmul(vp2[:, n2 * 512:(n2 + 1) * 512],
                                     encT[:, kt, t * P:(t + 1) * P],
                                     wv2[:, kt, n2 * 512:(n2 + 1) * 512],
                                     start=(kt == 0), stop=(kt == KT - 1))
            vt2 = v_pool.tile([P, D + 1], F32R, tag="vt", name=f"v2t_{t}")
            nc.scalar.copy(vt2[:, 0:D], vp2[:])
            nc.vector.tensor_copy(vt2[:, D:D + 1], ones_col[:])
            nc.tensor.matmul(G2_ps[:, 0:512], sigk2[:, t, :], vt2[:, 0:512],
                             start=(t == 0), stop=(t == T - 1))
            nc.tensor.matmul(G2_ps[:, 512:1024], sigk2[:, t, :],
                             vt2[:, 512:1024], start=(t == 0), stop=(t == T - 1))
            nc.tensor.matmul(G2_ps[:, 1024:1025],
                             sigk2[:, t, :].bitcast(F32),
                             vt2[:, D:D + 1].bitcast(F32),
                             start=(t == 0), stop=(t == T - 1))
        attn_state(G2_ps, v_psum, cc2_in, "kv2")
        if NO_CC:
            nc.sync.dma_start(cc2_out[:], cc2_in[:])
        else:
            nc.gpsimd.collective_compute(
                "AllReduce", ALU.add, replica_groups=[list(range(N_CORES))],
                ins=[cc2_in.opt()], outs=[cc2_out.opt()])
        wo1_pool.release()
        encT_pool.release()
        wv2_pool.release()
        v_pool.release()
        v_psum.release()
        G_psum.release()
        k_psum.release()
        ksc_pool.release()
        sigk_pool.release()

        # ================= PHASE B: attn1 + LN1 + q2 =================
        o2row_pool = pool("o2row", 5 if affine_trivial else 4)
        o2T_pool = pool("o2T", 1)
        hT_pool = pool("hT", 1)
        out1T_pool = pool("out1T", 1)
        out1T = out1T_pool.tile([P, KT, S_LOC], BF16, name="out1T")
        if affine_trivial:
            gbe1_pool = g1b = be1b = None
        else:
            gbe1_pool = pool("gbe1", 1)
            g1b = load_const("g1b", pl=gbe1_pool)
            be1b = load_const("be1b", pl=gbe1_pool)

        a_psum = pool("a_psum", 2, side="right", space="PSUM")
        tpB = pool("tpB", 2, space="PSUM")
        for t in range(T):
            ap_ = a_psum.tile([P, D], F32, tag="a", name=f"a1_{t}")
            xr2 = row_pool.tile([P, D], F32R, tag="xr", name=f"xr2_{t}")
            nc.sync.dma_start(xr2[:], d["x_loc"][t * P:(t + 1) * P, :])
            for n2 in range(2):
                sl = slice(n2 * 512, (n2 + 1) * 512)
                nc.tensor.matmul(ap_[:, sl], sigq1[:, t * P:(t + 1) * P],
                                 Ma1[:, sl], start=True, stop=False)
                nc.tensor.matmul(ap_[:, sl], identr[:], xr2[:, sl],
                                 start=False, stop=True)
            o1t = ln_pool.tile([P, D], F32R, tag="ot", name=f"o1t_{t}")
            layernorm(ap_, o1t[:], g1b, be1b, ln_pool, f"ln1_{t}")
            nc.sync.dma_start(out1d[t * P:(t + 1) * P, :], o1t[:])
            transpose_into(o1t.bitcast(F32), out1T, t, tpB, "split", f"o1{t}")
        tpB.release()

        q2p = pool("q2p", 2, space="PSUM")
        for n in range(S_LOC // 512):
            qp2 = q2p.tile([16, 512], F32, tag="qc", name=f"q2_{n}")
            for kt in range(KT):
                nc.tensor.matmul(qp2[:], wq2[:, kt, :],
                                 out1T[:, kt, n * 512:(n + 1) * 512],
                                 start=(kt == 0), stop=(kt == KT - 1))
            nc.scalar.activation(sigq2[0:16, n * 512:(n + 1) * 512], qp2[:],
                                 AF.Sigmoid, bias=bq2c[:])
        q2p.release()
        ma1_pool.release()
        sigq1_pool.release()
        row_pool.release()
        if gbe1_pool is not None:
            gbe1_pool.release()
        out1T_pool.release()

        # ---- m2 state chain ----
        ma2_pool = pool("ma2", 1)
        if affine_trivial:
            gbe2_pool = g2b = be2b = None
        else:
            gbe2_pool = pool("gbe2", 1)
            g2b = load_const("g2b", pl=gbe2_pool)
            be2b = load_const("be2b", pl=gbe2_pool)
        o1row_pool = pool("o1row", 2)
        wo2_pool = pool("wo2", 1)
        wo2 = wo2_pool.tile([P, KT, D], F32R, name="wo2")
        for kt in range(KT):
            nc.sync.dma_start(wo2[:, kt, :], d["wo2p"][:, kt, :])
        sps2 = pool("sps2", 1, space="PSUM")
        Ma2 = state_to_M(cc2_out, wo2, bv2h, bo2r, sps2, ma2_pool, "m2")
        sps2.release()
        wo2_pool.release()

        wf2_pool = pool("wf2", 1)
        wf2 = wf2_pool.tile([P, MT, D], BF16, name="wf2")
        for m in range(MT):
            nc.gpsimd.dma_start(wf2[:, m, :], d["wf2p"][:, m, :])
        if affine_trivial:
            gbe3_pool = g3b = be3b = None
        else:
            gbe3_pool = pool("gbe3", 1)
            g3b = load_const("g3b", pl=gbe3_pool)
            be3b = load_const("be3b", pl=gbe3_pool)
        ln3_pool = pool("ln3", 2)
        tpD = pool("tpD", 2, space="PSUM")
        h_psum = pool("h_psum", 2, space="PSUM")

        # ====== PHASE C+D: attn2+LN2 interleaved with FFN blocks ======
        def attn2_group(g):
            for t4 in range(TQ):
                t = g * TQ + t4
                ap2 = a_psum.tile([P, D], F32, tag="a", name=f"a2_{t}")
                o1r = o1row_pool.tile([P, D], F32R, tag="o1r",
                                      name=f"o1r_{t}")
                nc.sync.dma_start(o1r[:], out1d[t * P:(t + 1) * P, :])
                for n2 in range(2):
                    sl = slice(n2 * 512, (n2 + 1) * 512)
                    nc.tensor.matmul(ap2[:, sl],
                                     sigq2[:, t * P:(t + 1) * P],
                                     Ma2[:, sl], start=True, stop=False)
                    nc.tensor.matmul(ap2[:, sl], identr[:], o1r[:, sl],
                                     start=False, stop=True)
                o2t = ln_pool.tile([P, D], F32R, tag="ot", name=f"o2t_{t}")
                layernorm(ap2, o2t[:], g2b, be2b, ln_pool, f"ln2_{t}")
                nc.sync.dma_start(out2d[t * P:(t + 1) * P, :], o2t[:])

        def ffn_block(sq):
            """FFN for s-quarter sq.  Loads ride the idle GpSimd queue to
            dodge head-of-line blocking behind attn2's waits on sync."""
            o2T = o2T_pool.tile([P, KT, TQ * P], BF16, tag="o2T",
                                name=f"o2T_{sq}")
            o2rows = []
            for t4 in range(TQ):
                t = sq * TQ + t4
                o2r = o2row_pool.tile([P, D], F32R, tag="o2r",
                                      name=f"o2r_{t}")
                nc.gpsimd.dma_start(o2r[:], out2d[t * P:(t + 1) * P, :])
                o2rows.append(o2r)
                transpose_into(o2r.bitcast(F32), o2T, t4, tpD, "vector",
                               f"o2{t}")
            hT = hT_pool.tile([P, MT, TQ * P], BF16, tag="hT",
                              name=f"hT_{sq}")
            for m in range(MT):
                wf1m = wf1_pool.tile([P, KT, P], BF16, tag="wf1m",
                                     name=f"wf1_{sq}_{m}")
                nc.gpsimd.dma_start(wf1m[:], d["wf1p"][m])
                hp = h_psum.tile([P, TQ * P], F32, tag="hp",
                                 name=f"hp_{sq}_{m}")
                for kt in range(KT):
                    nc.tensor.matmul(hp[:], wf1m[:, kt, :], o2T[:, kt, :],
                                     start=(kt == 0), stop=(kt == KT - 1))
                nc.scalar.activation(hT[:, m, :], hp[:], AF.Relu,
                                     bias=bf1c[:, m:m + 1])
            for t4 in range(TQ):
                t = sq * TQ + t4
                o3 = ln3_pool.tile([P, D], F32, tag="o3f", name=f"o3f_{t}")
                st6 = ln3_pool.tile([P, 2, 6], F32, tag="st6",
                                    name=f"st6f_{t}")
                chunks = []
                for n2 in range(2):
                    sl = slice(n2 * 512, (n2 + 1) * 512)
                    op3 = h_psum.tile([P, 512], F32, tag="hp",
                                      name=f"o3c_{t}_{n2}")
                    for m in range(MT):
                        nc.tensor.matmul(op3[:],
                                         hT[:, m, t4 * P:(t4 + 1) * P],
                                         wf2[:, m, sl],
                                         start=(m == 0), stop=False)
                    nc.tensor.matmul(op3[:], identr[:], o2rows[t4][:, sl],
                                     start=False, stop=affine_trivial)
                    if not affine_trivial:
                        nc.tensor.matmul(op3[:], identr[:], bf2br[:, sl],
                                         start=False, stop=True)
                    nc.vector.bn_stats(st6[:, n2, :], op3[:])
                    chunks.append(op3)
                mv = ln3_pool.tile([P, 2], F32, tag="mv", name=f"mvf_{t}")
                nc.vector.bn_aggr(mv[:], st6[:])
                std = ln3_pool.tile([P, 1], F32, tag="std", name=f"stdf_{t}")
                nc.scalar.activation(std[:], mv[:, 1:2], AF.Sqrt,
                                     bias=eps[:])
                rstd = ln3_pool.tile([P, 1], F32, tag="rstd",
                                     name=f"rstdf_{t}")
                nc.vector.reciprocal(rstd[:], std[:])
                nmr = ln3_pool.tile([P, 1], F32, tag="nmr", name=f"nmrf_{t}")
                nc.vector.scalar_tensor_tensor(nmr[:], mv[:, 0:1], -1.0,
                                               rstd[:], op0=ALU.mult,
                                               op1=ALU.mult)
                for n2 in range(2):
                    sl = slice(n2 * 512, (n2 + 1) * 512)
                    nc.scalar.activation(o3[:, sl], chunks[n2][:],
                                         AF.Identity, bias=nmr[:],
                                         scale=rstd[:])
                if not affine_trivial:
                    nc.vector.scalar_tensor_tensor(o3[:], o3[:], 1.0,
                                                   g3b[:], op0=ALU.mult,
                                                   op1=ALU.mult)
                    nc.vector.tensor_add(o3[:], o3[:], be3b[:])
                nc.sync.dma_start(out_dram[t * P:(t + 1) * P, :], o3[:])

        for g in range(SQ):
            attn2_group(g)
            if g >= 1:
                ffn_block(g - 1)
        ffn_block(SQ - 1)

        a_psum.release()
        rel = [h_psum, tpD, ln3_pool]
        if gbe3_pool is not None:
            rel.append(gbe3_pool)
        rel.append(wf2_pool)
        rel.append(o1row_pool)
        if gbe2_pool is not None:
            rel.append(gbe2_pool)
        rel += [ma2_pool, hT_pool, o2T_pool, o2row_pool, wf1_pool,
                cpool, dram_pool, sigq2_pool, state_pool, ln_pool]
        for p_ in rel:
            p_.release()

    nc.compile()
    return nc


_NC_CACHE = {}


def _get_nc(affine_trivial):
    if affine_trivial not in _NC_CACHE:
        _NC_CACHE[affine_trivial] = build_program(affine_trivial)
    return _NC_CACHE[affine_trivial]


def _affine_trivial(inputs):
    for g in ("g1", "g2", "g3"):
        if not np.all(np.asarray(inputs[g]) == 1.0):
            return False
    for b in ("be1", "be2", "be3", "bf2"):
        if not np.all(np.asarray(inputs[b]) == 0.0):
            return False
    return True


def _prep_inputs(inputs):
    f32 = lambda a: np.ascontiguousarray(np.asarray(a, dtype=np.float32))
    x = f32(inputs["x"])
    enc = f32(inputs["enc"])

    def pack_w(w):  # [D, n] -> [P, KT, n]
        w = f32(w)
        return np.ascontiguousarray(w.reshape(KT, P, -1).transpose(1, 0, 2))

    shared = {
        "wq1p": pack_w(inputs["wq1"]), "wk1p": pack_w(inputs["wk1"]),
        "wk2p": pack_w(inputs["wk2"]),
        "wq2p": pack_w(inputs["wq2"]).astype(ml_dtypes.bfloat16),
        "wv1p": pack_w(inputs["wv1"]), "wo1p": pack_w(inputs["wo1"]),
        "wv2p": pack_w(inputs["wv2"]), "wo2p": pack_w(inputs["wo2"]),
    }
    wf1 = f32(inputs["wf1"])  # [D, FF]
    wf1p = wf1.reshape(KT, P, MT, P).transpose(2, 1, 0, 3)
    shared["wf1p"] = np.ascontiguousarray(wf1p.astype(ml_dtypes.bfloat16))
    wf2 = f32(inputs["wf2"])  # [FF, D]
    shared["wf2p"] = np.ascontiguousarray(
        wf2.reshape(MT, P, D).transpose(1, 0, 2).astype(ml_dtypes.bfloat16))

    def bcast(v):
        v = f32(v).reshape(-1)
        return np.ascontiguousarray(np.broadcast_to(v[None, :], (P, v.size)))

    shared["bq1c"] = f32(inputs["bq1"]).reshape(H, 1)
    shared["bq2c"] = f32(inputs["bq2"]).reshape(H, 1)
    shared["bk1b"] = bcast(inputs["bk1"])
    shared["bk2b"] = bcast(inputs["bk2"])
    shared["bv1h"] = f32(inputs["bv1"]).reshape(H, DV)
    shared["bv2h"] = f32(inputs["bv2"]).reshape(H, DV)
    shared["bo1r"] = np.ascontiguousarray(np.broadcast_to(f32(inputs["bo1"])[None, :], (H, D)))
    shared["bo2r"] = np.ascontiguousarray(np.broadcast_to(f32(inputs["bo2"])[None, :], (H, D)))
    shared["bf1c"] = np.ascontiguousarray(f32(inputs["bf1"]).reshape(MT, P).T)
    shared["bf2b"] = bcast(inputs["bf2"])
    for k_src, k_dst in [("g1", "g1b"), ("be1", "be1b"), ("g2", "g2b"),
                         ("be2", "be2b"), ("g3", "g3b"), ("be3", "be3b")]:
        shared[k_dst] = bcast(inputs[k_src])

    hh = np.arange(H)
    jj = np.arange(D)
    shared["maskh"] = (jj[None, :] // DV == hh[:, None]).astype(np.float32)
    pp = np.arange(P)
    kk = np.arange(KT)
    shared["maskT"] = ((kk[None, :, None] * P + pp[:, None, None]) // DV
                       == hh[None, None, :]).astype(np.float32)
    shared["U16"] = (hh[:, None] <= hh[None, :]).astype(np.float32)

    in_maps = []
    p64 = np.arange(64)
    for c in range(N_CORES):
        b, half = c // 2, c % 2
        s0 = half * S_LOC
        m = dict(shared)
        m["x_loc"] = np.ascontiguousarray(x[b, s0:s0 + S_LOC, :])
        m["enc_loc"] = np.ascontiguousarray(enc[b, s0:s0 + S_LOC, :])
        bsel = (p64[:, None] == 16 * b + hh[None, :]).astype(np.float32)
        m["Bsel"] = bsel
        m["BselT"] = np.ascontiguousarray(bsel.T)
        in_maps.append(m)
    return in_maps


def run_on_hw(inputs, **kwargs):
    nc = _get_nc(_affine_trivial(inputs))
    in_maps = _prep_inputs(inputs)
    return run_bass_kernel_spmd(nc, in_maps, list(range(N_CORES)), **kwargs)


def kernel(**inputs):
    r = run_on_hw(inputs)
    out = np.empty((B, S, D), dtype=np.float32)
    for c in range(N_CORES):
        b, half = c // 2, c % 2
        out[b, half * S_LOC:(half + 1) * S_LOC, :] = r.results[c]["out_loc"]
    return (out, np.zeros_like(out), np.zeros_like(out))



# revision 11
# speedup vs baseline: 1.3378x; 1.3378x over previous
"""Trainium2 Bass kernel for nn_DecoderLayer_43877385896448 (see spec).

Decoder layer with sigmoid linear attention (rank-1 per head), 2 attn blocks,
FFN, 3 layernorms.  B=4, S=4096, D=1024, H=16 heads (depth-1 q/k per head),
F=4096.

Sharding: rows (b, s) split across 8 cores -> core c owns batch b=c//2,
sequence half (c%2)*2048.  Row-parallel matmuls with replicated weights; the
only cross-core exchange is an AllReduce of the per-batch attention state
kv[16,65] per attention block, overlapped with compute.

Key algebra:
 - kv state: kv = sigk^T (X wv + bv) = ((sigk^T X) wv) + rowsum(sigk) bv, so
   the [S,D]x[D,D] v-projection collapses to a [16,D]x[D,D] after a cheap
   [16,S]x[S,D] accumulation that rides the row tiles (no v materialized).
 - attn out = [sigq ; 1] @ M_aug with M_aug = [blockdiag(cumsum kv) wo ; bo],
   eliminating the [S,D]x[D,D] output projection.
Weights on the fat paths are bf16 (q/k/v/o projections feed sigmoids / tiny
states; FFN measured 2e-3 rel err); residual adds ride the PE via identity
matmuls so PSUM drains fast and the PE HAM clock stays warm.
"""

import numpy as np
import ml_dtypes

import concourse.bass as bass
import concourse.bacc as bacc
import concourse.tile as tile
import concourse.mybir as mybir
from concourse import masks
from concourse.bass_utils import run_bass_kernel_spmd

F32 = mybir.dt.float32
F32R = mybir.dt.float32r
BF16 = mybir.dt.bfloat16
AF = mybir.ActivationFunctionType
ALU = mybir.AluOpType
AX = mybir.AxisListType

B, S, D, H, FF = 4, 4096, 1024, 16, 4096
DV = D // H            # 64
P = 128
N_CORES = 8
S_LOC = 2048           # rows per core
T = S_LOC // P         # 16 s-tiles per core
KT = D // P            # 8 k-tiles over D
MT = FF // P           # 32 dff tiles
EPS = 1e-6
SQ = 4                 # ffn processes s in 4 quarters of 512 rows
TQ = T // SQ
import os
NO_CC = bool(int(os.environ.get("BASS_NO_CC", "0")))


def build_program(affine_trivial=False):
    nc = bacc.Bacc("TRN2", target_bir_lowering=False, debug=False,
                   num_devices=N_CORES)

    d = {}

    def din(name, shape, dtype=F32):
        d[name] = nc.dram_tensor(name, list(shape), dtype,
                                 kind="ExternalInput").ap()

    din("x_loc", [S_LOC, D], BF16)
    din("enc_loc", [S_LOC, D], BF16)
    for w in ["wq1p", "wk1p", "wq2p", "wk2p"]:
        din(w, [P, KT, H], BF16)
    for w in ["wv1p", "wo1p", "wv2p", "wo2p"]:
        din(w, [P, KT, D], BF16)
    din("wf1p", [MT, P, KT, P], BF16)
    din("wf2p", [P, MT, D], BF16)
    din("bq1c", [H, 1]); din("bq2c", [H, 1])
    din("bk1b", [P, H]); din("bk2b", [P, H])
    din("bv1h", [H, DV]); din("bv2h", [H, DV])
    din("bo1r", [H + 1, D], BF16); din("bo2r", [H + 1, D], BF16)
    din("bf1c", [P, MT]); din("bf2b", [P, D], F32R)
    for v in ["g1b", "be1b", "g2b", "be2b", "g3b", "be3b"]:
        din(v, [P, D])
    din("maskh", [H, D]); din("maskT", [P, KT, H + 1])
    din("U16", [H, H]); din("Bsel", [64, H]); din("BselT", [H, 64])
    out_dram = nc.dram_tensor("out_loc", [S_LOC, D], F32,
                              kind="ExternalOutput").ap()

    with tile.TileContext(nc) as tc:
        def pool(name, bufs, side="left", space="SBUF"):
            return tc.alloc_tile_pool(name=name, bufs=bufs, side=side,
                                      space=space)

        # ---------------- constants ----------------
        cpool = pool("consts", 1)
        ident = cpool.tile([P, P], F32, name="ident")
        masks.make_identity(nc, ident[:])
        identr = cpool.tile([P, P], F32R, name="identr")
        nc.vector.tensor_copy(identr[:], ident[:])
        identb = cpool.tile([P, P], BF16, name="identb")
        nc.vector.tensor_copy(identb[:], ident[:])

        def load_const(key, dtype=F32, pl=None):
            pl = pl if pl is not None else cpool
            t_ = pl.tile([int(s) for s in d[key].shape], dtype,
                         name=f"c_{key}")
            nc.sync.dma_start(t_[:], d[key][:])
            return t_

        maskh = load_const("maskh")
        maskT = load_const("maskT")
        U16 = load_const("U16")
        Bsel = load_const("Bsel")
        BselT = load_const("BselT")
        bq1c = load_const("bq1c"); bq2c = load_const("bq2c")
        bk1b = load_const("bk1b"); bk2b = load_const("bk2b")
        bv1h = load_const("bv1h"); bv2h = load_const("bv2h")
        bo1r = load_const("bo1r", BF16); bo2r = load_const("bo2r", BF16)
        bf1c = load_const("bf1c")
        bf2br = load_const("bf2b", F32R)
        wq1 = load_const("wq1p", BF16); wk1 = load_const("wk1p", BF16)
        wq2 = load_const("wq2p", BF16); wk2 = load_const("wk2p", BF16)
        eps = cpool.tile([P, 1], F32, name="epsc")
        nc.vector.memset(eps[:], EPS)
        ones_col = cpool.tile([P, 1], F32, name="ones_col")
        nc.vector.memset(ones_col[:], 1.0)
        ones_colb = cpool.tile([P, 1], BF16, name="ones_colb")
        nc.vector.memset(ones_colb[:], 1.0)

        dram_pool = pool("ccdram", 1, space="DRAM")
        cc1_in = dram_pool.tile([64, 65], F32, name="cc1_in")
        cc1_out = dram_pool.tile([64, 65], F32, name="cc1_out")
        cc2_in = dram_pool.tile([64, 65], F32, name="cc2_in")
        cc2_out = dram_pool.tile([64, 65], F32, name="cc2_out")
        out1d = dram_pool.tile([S_LOC, D], F32R, name="out1d")

        # bigw: one 64KB/partition slot shared (in time) by wo1+wo2 during
        # phases A/B, then wf2 during the FFN (tag-shared, LIFO-friendly)
        bigw_pool = pool("bigw", 1)
        wo12 = bigw_pool.tile([P, 2 * KT, D], BF16, tag="w", name="wo12")
        for kt in range(KT):
            nc.sync.dma_start(wo12[:, kt, :], d["wo1p"][:, kt, :])
        for kt in range(KT):
            nc.gpsimd.dma_start(wo12[:, KT + kt, :], d["wo2p"][:, kt, :])

        # ---------------- long-lived left pools ----------------
        xrows_pool = pool("xrows", 1)
        xrows = xrows_pool.tile([P, T, D], BF16, name="xrows")
        xT_pool = pool("xT", 1)
        xT = xT_pool.tile([P, KT, S_LOC], BF16, name="xT")
        row_pool = pool("rows", 2)
        encT_pool = pool("encT", 2)
        wvs_pool = pool("wvs", 2)

        # ---------------- right pools (live whole program) ----------------
        sigk_pool = pool("sigk", 1, side="right")
        sigq1_pool = pool("sigq1", 1, side="right")
        sigq2_pool = pool("sigq2", 1, side="right")
        state_pool = pool("state", 1, side="right")
        ma1_pool = pool("ma1", 1, side="right")
        ma2_pool = pool("ma2", 1, side="right")
        ln_pool = pool("ln", 2, side="right")
        if affine_trivial:
            g1b = be1b = g2b = be2b = g3b = be3b = None
        else:
            gbe_pool = pool("gbe", 1, side="right")
            g1b = load_const("g1b", pl=gbe_pool)
            be1b = load_const("be1b", pl=gbe_pool)
            g2b = load_const("g2b", pl=gbe_pool)
            be2b = load_const("be2b", pl=gbe_pool)
            g3b = load_const("g3b", pl=gbe_pool)
            be3b = load_const("be3b", pl=gbe_pool)

        sigk1 = sigk_pool.tile([P, T, H], BF16, name="sigk1")
        sigk2 = sigk_pool.tile([P, T, H], BF16, name="sigk2")
        sigq1 = sigq1_pool.tile([H + 1, S_LOC], F32R, name="sigq1")
        sigq2 = sigq2_pool.tile([H + 1, S_LOC], F32R, name="sigq2")
        # row H stays 1.0 (the M_aug bias row); sigmoids overwrite rows 0:H
        nc.vector.memset(sigq1[:].bitcast(F32), 1.0)
        nc.vector.memset(sigq2[:].bitcast(F32), 1.0)

        # ---------------- helpers ----------------
        def transpose_into(src_ap, dst, col0, tp_pool, nm, bf=False):
            """src [128, D] row tile -> dst[:, kt, col0:col0+128] (transposed).
            bf=True: src/dst bf16 path (tp psum in bf16)."""
            for half in range(2):
                if bf:
                    tp = tp_pool.tile([P, 512], BF16, tag="tpb",
                                      name=f"tpb_{nm}_{half}")
                    idm = identb
                else:
                    tp = tp_pool.tile([P, 512], F32, tag="tp",
                                      name=f"tp_{nm}_{half}")
                    idm = ident
                for j in range(4):
                    kt = half * 4 + j
                    nc.tensor.matmul(tp[:, j * P:(j + 1) * P],
                                     src_ap[:, kt * P:(kt + 1) * P],
                                     idm[:], is_transpose=True)
                dst_ap = dst[:, half * 4:(half + 1) * 4, col0:col0 + P]
                src_t = tp[:].rearrange("p (k n) -> p k n", k=4)
                if half == 0:
                    nc.vector.tensor_copy(dst_ap, src_t)
                else:
                    nc.scalar.copy(dst_ap, src_t)

        def ln_psum(chunks, outs, g_sb, be_sb, lnp, nm):
            """LayerNorm from psum chunks (total width D) -> outs dst APs.
            bn_stats is capped at 512 free elems; split wider chunks."""
            pieces = []
            for c in chunks:
                w = c.shape[-1]
                if w > 512:
                    for j in range(0, w, 512):
                        pieces.append(c[:, j:j + 512])
                else:
                    pieces.append(c)
            st6 = lnp.tile([P, len(pieces), 6], F32, tag="st6",
                           name=f"st6_{nm}")
            for i, c in enumerate(pieces):
                nc.vector.bn_stats(st6[:, i, :], c)
            mv = lnp.tile([P, 2], F32, tag="mv", name=f"mv_{nm}")
            nc.vector.bn_aggr(mv[:], st6[:])
            std = lnp.tile([P, 1], F32, tag="std", name=f"std_{nm}")
            nc.scalar.activation(std[:], mv[:, 1:2], AF.Sqrt, bias=eps[:])
            rstd = lnp.tile([P, 1], F32, tag="rstd", name=f"rstd_{nm}")
            nc.vector.reciprocal(rstd[:], std[:])
            nmr = lnp.tile([P, 1], F32, tag="nmr", name=f"nmr_{nm}")
            nc.vector.scalar_tensor_tensor(nmr[:], mv[:, 0:1], -1.0, rstd[:],
                                           op0=ALU.mult, op1=ALU.mult)
            off = 0
            for i, c in enumerate(chunks):
                w = c.shape[-1]
                if affine_trivial:
                    nc.scalar.activation(outs[i], c, AF.Identity,
                                         bias=nmr[:], scale=rstd[:])
                else:
                    sl = slice(off, off + w)
                    xh = lnp.tile([P, w], F32, tag="xh", name=f"xh_{nm}_{i}")
                    nc.scalar.activation(xh[:], c, AF.Identity,
                                         bias=nmr[:], scale=rstd[:])
                    nc.vector.scalar_tensor_tensor(xh[:], xh[:], 1.0,
                                                   g_sb[:, sl],
                                                   op0=ALU.mult, op1=ALU.mult)
                    nc.vector.tensor_add(outs[i], xh[:], be_sb[:, sl])
                off += w

        def kv_pack(A_ps, wv_key, sel_pool, cc_in, nm):
            """A_ps [16,1025] psum (sigk^T [X | 1]) -> G halves -> kv[16,65]
            -> batch-slot select [64,65] -> DMA for AllReduce."""
            asb = state_pool.tile([H, D + 1], F32, tag="asb",
                                  name=f"asb_{nm}")
            nc.vector.tensor_copy(asb[:], A_ps[:])
            atp = sel_pool.tile([P, KT * H], F32, tag="qc", name=f"atp_{nm}")
            for kt in range(KT):
                nc.tensor.matmul(atp[:, kt * H:(kt + 1) * H],
                                 asb[:, kt * P:(kt + 1) * P],
                                 ident[:H, :H], is_transpose=True)
            aT = state_pool.tile([P, KT, H], BF16, tag="aT", name=f"aT_{nm}")
            nc.vector.tensor_copy(aT[:],
                                  atp[:].rearrange("p (k h) -> p k h", k=KT))
            Ghs = [sel_pool.tile([H, 512], F32, tag="qc",
                                 name=f"G_{nm}_{half}") for half in range(2)]
            for kt in range(KT):
                wvc = wvs_pool.tile([P, D], BF16, tag="wv",
                                    name=f"wv_{nm}_{kt}")
                nc.scalar.dma_start(wvc[:], d[wv_key][:, kt, :])
                for half in range(2):
                    sl = slice(half * 512, (half + 1) * 512)
                    nc.tensor.matmul(Ghs[half][:], aT[:, kt, :], wvc[:, sl],
                                     start=(kt == 0), stop=(kt == KT - 1))
            kvph = []
            for half in range(2):
                sl = slice(half * 512, (half + 1) * 512)
                gt = state_pool.tile([H, 512], F32, tag="gt",
                                     name=f"gt_{nm}_{half}")
                nc.vector.tensor_mul(gt[:], Ghs[half][:], maskh[:, sl])
                kp = state_pool.tile([H, DV], F32, tag=f"kvp{half}",
                                     name=f"kvp_{nm}_{half}")
                nc.vector.tensor_reduce(
                    kp[:], gt[:].rearrange("p (c v) -> p v c", v=DV),
                    axis=AX.X, op=ALU.add)
                kvph.append(kp)
            kvp = state_pool.tile([H, DV + 1], F32, tag="kv",
                                  name=f"kv_{nm}")
            nc.vector.tensor_add(kvp[:, 0:DV], kvph[0][:], kvph[1][:])
            nc.vector.tensor_copy(kvp[:, DV:DV + 1], A_ps[:, D:D + 1])
            kvsel_ps = sel_pool.tile([64, 65], F32, tag="qc",
                                     name=f"kvselp_{nm}")
            nc.tensor.matmul(kvsel_ps[:], BselT[:], kvp[:],
                             start=True, stop=True)
            kvsel = state_pool.tile([64, 65], F32, tag="kvsel",
                                    name=f"kvsel_{nm}")
            nc.vector.tensor_copy(kvsel[:], kvsel_ps[:])
            nc.sync.dma_start(cc_in[:], kvsel[:])

        def state_to_M(cc_out, wo_off, bvh, bor, spsum, ma_pool, nm):
            """AllReduce out -> own-batch kv -> cumsum over heads ->
            M_aug [17,1024] (rows 0:16 blockdiag(cumsum kv) @ wo, row 16 bo)."""
            kvred = state_pool.tile([64, 65], F32, tag="kvred",
                                    name=f"kvred_{nm}")
            nc.sync.dma_start(kvred[:], cc_out[:])
            kvmy_ps = spsum.tile([H, 65], F32, tag="qc", name=f"kvmyp_{nm}")
            nc.tensor.matmul(kvmy_ps[:], Bsel[:], kvred[:],
                             start=True, stop=True)
            kvmy = state_pool.tile([H, 65], F32, tag="kvmy",
                                   name=f"kvmy_{nm}")
            nc.vector.tensor_copy(kvmy[:], kvmy_ps[:])
            kv_bv = state_pool.tile([H, DV], F32, tag="kv_bv",
                                    name=f"kv_bv_{nm}")
            nc.vector.scalar_tensor_tensor(
                kv_bv[:], bvh[:], kvmy[:, DV:DV + 1], kvmy[:, 0:DV],
                op0=ALU.mult, op1=ALU.add)
            scum_ps = spsum.tile([H, DV], F32, tag="qc", name=f"scump_{nm}")
            nc.tensor.matmul(scum_ps[:], U16[:], kv_bv[:],
                             start=True, stop=True)
            scum = state_pool.tile([H, DV], F32, tag="scumsb",
                                   name=f"scum_{nm}")
            nc.vector.tensor_copy(scum[:], scum_ps[:])
            scumT_ps = spsum.tile([DV, H], F32, tag="qc", name=f"scumTp_{nm}")
            nc.tensor.matmul(scumT_ps[:], scum[:], ident[:H, :H],
                             is_transpose=True)
            scumT2 = state_pool.tile([P, H + 1], F32, tag="scumT2",
                                     name=f"scumT2_{nm}")
            nc.vector.memset(scumT2[:, H:H + 1], 0.0)
            nc.vector.tensor_copy(scumT2[0:DV, 0:H], scumT_ps[:])
            nc.vector.tensor_copy(scumT2[DV:P, 0:H], scumT_ps[:])
            bdts = []
            for kt in range(KT):
                bdt = state_pool.tile([P, H + 1], BF16, tag=f"bdt{kt}",
                                      name=f"bdt_{nm}_{kt}")
                nc.vector.tensor_mul(bdt[:], scumT2[:], maskT[:, kt, :])
                bdts.append(bdt)
            Ma = ma_pool.tile([H + 1, D], F32R, name=f"Ma_{nm}")
            for n2 in range(2):
                sl = slice(n2 * 512, (n2 + 1) * 512)
                M_ps = spsum.tile([H + 1, 512], F32, tag="qc",
                                  name=f"M_{nm}_{n2}")
                nc.tensor.matmul(M_ps[:], identb[:H + 1, :H + 1], bor[:, sl],
                                 start=True, stop=False)
                for kt in range(KT):
                    nc.tensor.matmul(M_ps[:], bdts[kt][:],
                                     wo12[:, wo_off + kt, sl],
                                     start=False, stop=(kt == KT - 1))
                nc.vector.tensor_copy(Ma[:, sl], M_ps[:])
            return Ma

        # ================= PHASE A1: x side =================
        tpA = pool("tpA", 2, space="PSUM")
        k_psum = pool("k_psum", 1, space="PSUM")
        A_psum = pool("A_psum", 1, space="PSUM")
        q1p = pool("q1p", 2, space="PSUM")

        A1_ps = A_psum.tile([H, D + 1], F32, tag="A", name="A1_ps")
        for t in range(T):
            eng = nc.sync if t % 2 == 0 else nc.gpsimd
            eng.dma_start(xrows[:, t, :], d["x_loc"][t * P:(t + 1) * P, :])
            transpose_into(xrows[:, t, :], xT, t * P, tpA, f"x{t}", bf=True)
            kp = k_psum.tile([P, H], F32, tag="kp", name=f"k1p_{t}")
            for kt in range(KT):
                nc.tensor.matmul(kp[:], xT[:, kt, t * P:(t + 1) * P],
                                 wk1[:, kt, :],
                                 start=(kt == 0), stop=(kt == KT - 1))
            ktmp = state_pool.tile([P, H], F32, tag="ktmp", name=f"k1t_{t}")
            nc.vector.tensor_add(ktmp[:], kp[:], bk1b[:])
            nc.scalar.activation(sigk1[:, t, :], ktmp[:], AF.Sigmoid)
            for n2 in range(2):
                sl = slice(n2 * 512, (n2 + 1) * 512)
                nc.tensor.matmul(A1_ps[:, sl], sigk1[:, t, :],
                                 xrows[:, t, sl],
                                 start=(t == 0), stop=(t == T - 1))
            nc.tensor.matmul(A1_ps[:, D:D + 1],
                             sigk1[:, t, :], ones_colb[:],
                             start=(t == 0), stop=(t == T - 1))
            if t % 4 == 3:
                n = t // 4
                qp = q1p.tile([H, 512], F32, tag="qc", name=f"q1_{n}")
                for kt in range(KT):
                    nc.tensor.matmul(qp[:], wq1[:, kt, :],
                                     xT[:, kt, n * 512:(n + 1) * 512],
                                     start=(kt == 0), stop=(kt == KT - 1))
                nc.scalar.activation(sigq1[0:H, n * 512:(n + 1) * 512],
                                     qp[:], AF.Sigmoid, bias=bq1c[:])

        kv_pack(A1_ps, "wv1p", q1p, cc1_in, "kv1")
        if NO_CC:
            nc.sync.dma_start(cc1_out[:], cc1_in[:])
        else:
            nc.gpsimd.collective_compute(
                "AllReduce", ALU.add, replica_groups=[list(range(N_CORES))],
                ins=[cc1_in.opt()], outs=[cc1_out.opt()])

        # ================= PHASE A2: enc side (overlaps AllReduce 1) ====
        A2_ps = A_psum.tile([H, D + 1], F32, tag="A", name="A2_ps")
        for t in range(T):
            eng = nc.scalar if t % 2 == 0 else nc.gpsimd
            er = row_pool.tile([P, D], BF16, tag="er", name=f"er_{t}")
            eng.dma_start(er[:], d["enc_loc"][t * P:(t + 1) * P, :])
            eT = encT_pool.tile([P, KT, P], BF16, tag="eT", name=f"eT_{t}")
            transpose_into(er[:], eT, 0, tpA, f"e{t}", bf=True)
            kp2 = k_psum.tile([P, H], F32, tag="kp", name=f"k2p_{t}")
            for kt in range(KT):
                nc.tensor.matmul(kp2[:], eT[:, kt, :], wk2[:, kt, :],
                                 start=(kt == 0), stop=(kt == KT - 1))
            ktmp2 = state_pool.tile([P, H], F32, tag="ktmp", name=f"k2t_{t}")
            nc.vector.tensor_add(ktmp2[:], kp2[:], bk2b[:])
            nc.scalar.activation(sigk2[:, t, :], ktmp2[:], AF.Sigmoid)
            for n2 in range(2):
                sl = slice(n2 * 512, (n2 + 1) * 512)
                nc.tensor.matmul(A2_ps[:, sl], sigk2[:, t, :], er[:, sl],
                                 start=(t == 0), stop=(t == T - 1))
            nc.tensor.matmul(A2_ps[:, D:D + 1],
                             sigk2[:, t, :], ones_colb[:],
                             start=(t == 0), stop=(t == T - 1))

        # m1 state chain: fills gaps while enc-side work runs; waits AR1
        Ma1 = state_to_M(cc1_out, 0, bv1h, bo1r, q1p, ma1_pool, "m1")

        kv_pack(A2_ps, "wv2p", q1p, cc2_in, "kv2")
        if NO_CC:
            nc.sync.dma_start(cc2_out[:], cc2_in[:])
        else:
            nc.gpsimd.collective_compute(
                "AllReduce", ALU.add, replica_groups=[list(range(N_CORES))],
                ins=[cc2_in.opt()], outs=[cc2_out.opt()])

        wvs_pool.release()
        encT_pool.release()
        row_pool.release()
        q1p.release()
        A_psum.release()
        k_psum.release()
        tpA.release()

        # ================= PHASE B: attn1 + LN1 + q2 =================
        ab_psum = pool("ab_psum", 2, space="PSUM")
        tpB = pool("tpB", 2, space="PSUM", side="right")
        q2p = pool("q2p", 2, space="PSUM", side="right")
        roll_pool = pool("roll", 2)

        roll = None
        for t in range(T):
            if t % 4 == 0:
                roll = roll_pool.tile([P, KT, 512], BF16, tag="roll",
                                      name=f"roll_{t // 4}")
            ap_ = ab_psum.tile([P, D], F32, tag="a", name=f"a1_{t}")
            for n2 in range(2):
                sl = slice(n2 * 512, (n2 + 1) * 512)
                nc.tensor.matmul(ap_[:, sl], sigq1[:, t * P:(t + 1) * P],
                                 Ma1[:, sl], start=True, stop=False)
                nc.tensor.matmul(ap_[:, sl], identb[:], xrows[:, t, sl],
                                 start=False, stop=True)
            o1t = ln_pool.tile([P, D], F32R, tag="ot", name=f"o1t_{t}")
            ln_psum([ap_[:]], [o1t[:]], g1b, be1b, ln_pool, f"ln1_{t}")
            eng = nc.sync if t % 2 == 0 else nc.gpsimd
            eng.dma_start(out1d[t * P:(t + 1) * P, :], o1t[:])
            transpose_into(o1t[:].bitcast(F32), roll, (t % 4) * P, tpB,
                           f"o1{t}")
            if t % 4 == 3:
                n = t // 4
                qp2 = q2p.tile([H, 512], F32, tag="qc2", name=f"q2_{n}")
                for kt in range(KT):
                    nc.tensor.matmul(qp2[:], wq2[:, kt, :], roll[:, kt, :],
                                     start=(kt == 0), stop=(kt == KT - 1))
                nc.scalar.activation(sigq2[0:H, n * 512:(n + 1) * 512],
                                     qp2[:], AF.Sigmoid, bias=bq2c[:])

        roll_pool.release()
        q2p.release()
        tpB.release()
        ab_psum.release()
        xT_pool.release()
        xrows_pool.release()

        # ================= PHASE C+D: attn2+LN2 + FFN =================
        sps2 = pool("sps2", 1, space="PSUM")
        a2_psum = pool("a2_psum", 3, space="PSUM", side="right")
        tpD = pool("tpD", 2, space="PSUM", side="right")

        Ma2 = state_to_M(cc2_out, KT, bv2h, bo2r, sps2, ma2_pool, "m2")
        sps2.release()
        h_psum = pool("h_psum", 3, space="PSUM")

        wf2t = bigw_pool.tile([P, MT, D], BF16, tag="w", name="wf2")
        for m in range(MT):
            nc.scalar.dma_start(wf2t[:, m, :], d["wf2p"][:, m, :])

        def wf2_ap(m, sl):
            return wf2t[:, m, sl]

        hT_pool = pool("hT", 1)
        o2T_pool = pool("o2T", 1)
        o2row_pool = pool("o2row", 2)
        wf1_pool = pool("wf1", 4)
        o1row_pool = pool("o1row", 2)
        o3_pool = pool("o3", 2)

        o2qs = [None] * SQ

        def attn2_group(g):
            o2q = o2row_pool.tile([P, TQ * D], BF16, tag="o2q",
                                  name=f"o2q_{g}")
            o2qs[g] = o2q
            for t4 in range(TQ):
                t = g * TQ + t4
                o1r = o1row_pool.tile([P, D], F32R, tag="o1r",
                                      name=f"o1r_{t}")
                nc.gpsimd.dma_start(o1r[:], out1d[t * P:(t + 1) * P, :])
                chunks, outs = [], []
                for n2 in range(2):
                    sl = slice(n2 * 512, (n2 + 1) * 512)
                    apc = a2_psum.tile([P, 512], F32, tag="a2",
                                       name=f"a2_{t}_{n2}")
                    nc.tensor.matmul(apc[:], sigq2[:, t * P:(t + 1) * P],
                                     Ma2[:, sl], start=True, stop=False)
                    nc.tensor.matmul(apc[:], identr[:], o1r[:, sl],
                                     start=False, stop=True)
                    chunks.append(apc[:])
                    outs.append(o2q[:, t4 * D + n2 * 512:t4 * D + (n2 + 1) * 512])
                ln_psum(chunks, outs, g2b, be2b, ln_pool, f"ln2_{t}")

        def ffn_block(g):
            o2q = o2qs[g]
            o2T = o2T_pool.tile([P, KT, TQ * P], BF16, tag="o2T",
                                name=f"o2T_{g}")
            for t4 in range(TQ):
                src = o2q[:, t4 * D:(t4 + 1) * D]
                transpose_into(src, o2T, t4 * P, tpD, f"o2{g}_{t4}", bf=True)
            hT = hT_pool.tile([P, MT, TQ * P], BF16, tag="hT",
                              name=f"hT_{g}")
            for m in range(MT):
                wf1m = wf1_pool.tile([P, KT, P], BF16, tag="wf1m",
                                     name=f"wf1_{g}_{m}")
                eng = nc.gpsimd if m % 2 == 0 else nc.sync
                eng.dma_start(wf1m[:], d["wf1p"][m])
                hp = h_psum.tile([P, TQ * P], F32, tag="hp",
                                 name=f"hp_{g}_{m}")
                for kt in range(KT):
                    nc.tensor.matmul(hp[:], wf1m[:, kt, :], o2T[:, kt, :],
                                     start=(kt == 0), stop=(kt == KT - 1))
                nc.scalar.activation(hT[:, m, :], hp[:], AF.Relu,
                                     bias=bf1c[:, m:m + 1])
            for t4 in range(TQ):
                t = g * TQ + t4
                o3 = o3_pool.tile([P, D], F32, tag="o3", name=f"o3_{t}")
                chunks, outs = [], []
                for n2 in range(2):
                    sl = slice(n2 * 512, (n2 + 1) * 512)
                    op3 = h_psum.tile([P, 512], F32, tag="hp",
                                      name=f"o3c_{t}_{n2}")
                    for m in range(MT):
                        nc.tensor.matmul(op3[:],
                                         hT[:, m, t4 * P:(t4 + 1) * P],
                                         wf2_ap(m, sl),
                                         start=(m == 0), stop=False)
                    nc.tensor.matmul(op3[:], identb[:],
                                     o2q[:, t4 * D + n2 * 512:
                                          t4 * D + (n2 + 1) * 512],
                                     start=False, stop=affine_trivial)
                    if not affine_trivial:
                        nc.tensor.matmul(op3[:], identr[:], bf2br[:, sl],
                                         start=False, stop=True)
                    chunks.append(op3[:])
                    outs.append(o3[:, sl])
                ln_psum(chunks, outs, g3b, be3b, ln_pool, f"ln3_{t}")
                nc.sync.dma_start(out_dram[t * P:(t + 1) * P, :], o3[:])

        for g in range(SQ):
            attn2_group(g)
            if g >= 1:
                ffn_block(g - 1)
        ffn_block(SQ - 1)

        for p_ in [o3_pool, o1row_pool, wf1_pool, o2row_pool, o2T_pool,
                   hT_pool, h_psum, tpD, a2_psum,
                   bigw_pool, ln_pool, ma2_pool, ma1_pool,
                   state_pool, sigq2_pool, sigq1_pool, sigk_pool,
                   dram_pool, cpool]:
            p_.release()
        if not affine_trivial:
            gbe_pool.release()

    nc.compile()
    return nc


_NC_CACHE = {}


def _get_nc(affine_trivial):
    if affine_trivial not in _NC_CACHE:
        _NC_CACHE[affine_trivial] = build_program(affine_trivial)
    return _NC_CACHE[affine_trivial]


def _affine_trivial(inputs):
    for g in ("g1", "g2", "g3"):
        if not np.all(np.asarray(inputs[g]) == 1.0):
            return False
    for b in ("be1", "be2", "be3", "bf2"):
        if not np.all(np.asarray(inputs[b]) == 0.0):
            return False
    return True


def _prep_inputs(inputs):
    f32 = lambda a: np.ascontiguousarray(np.asarray(a, dtype=np.float32))
    bf = lambda a: np.ascontiguousarray(
        np.asarray(a, dtype=np.float32).astype(ml_dtypes.bfloat16))
    x = f32(inputs["x"])
    enc = f32(inputs["enc"])

    def pack_w(w):  # [D, n] -> [P, KT, n]
        w = f32(w)
        return np.ascontiguousarray(w.reshape(KT, P, -1).transpose(1, 0, 2))

    shared = {
        "wq1p": bf(pack_w(inputs["wq1"])), "wk1p": bf(pack_w(inputs["wk1"])),
        "wq2p": bf(pack_w(inputs["wq2"])), "wk2p": bf(pack_w(inputs["wk2"])),
        "wv1p": bf(pack_w(inputs["wv1"])), "wo1p": bf(pack_w(inputs["wo1"])),
        "wv2p": bf(pack_w(inputs["wv2"])), "wo2p": bf(pack_w(inputs["wo2"])),
    }
    wf1 = f32(inputs["wf1"])  # [D, FF]
    wf1p = wf1.reshape(KT, P, MT, P).transpose(2, 1, 0, 3)
    shared["wf1p"] = np.ascontiguousarray(wf1p.astype(ml_dtypes.bfloat16))
    wf2 = f32(inputs["wf2"])  # [FF, D]
    shared["wf2p"] = np.ascontiguousarray(
        wf2.reshape(MT, P, D).transpose(1, 0, 2).astype(ml_dtypes.bfloat16))

    def bcast(v):
        v = f32(v).reshape(-1)
        return np.ascontiguousarray(np.broadcast_to(v[None, :], (P, v.size)))

    def bo_aug(v):  # [17, D]: rows 0:16 zero, row 16 = bo
        m = np.zeros((H + 1, D), np.float32)
        m[H, :] = f32(v).reshape(-1)
        return np.ascontiguousarray(m.astype(ml_dtypes.bfloat16))

    shared["bq1c"] = f32(inputs["bq1"]).reshape(H, 1)
    shared["bq2c"] = f32(inputs["bq2"]).reshape(H, 1)
    shared["bk1b"] = bcast(inputs["bk1"])
    shared["bk2b"] = bcast(inputs["bk2"])
    shared["bv1h"] = f32(inputs["bv1"]).reshape(H, DV)
    shared["bv2h"] = f32(inputs["bv2"]).reshape(H, DV)
    shared["bo1r"] = bo_aug(inputs["bo1"])
    shared["bo2r"] = bo_aug(inputs["bo2"])
    shared["bf1c"] = np.ascontiguousarray(f32(inputs["bf1"]).reshape(MT, P).T)
    shared["bf2b"] = bcast(inputs["bf2"])
    for k_src, k_dst in [("g1", "g1b"), ("be1", "be1b"), ("g2", "g2b"),
                         ("be2", "be2b"), ("g3", "g3b"), ("be3", "be3b")]:
        shared[k_dst] = bcast(inputs[k_src])

    hh = np.arange(H)
    jj = np.arange(D)
    shared["maskh"] = (jj[None, :] // DV == hh[:, None]).astype(np.float32)
    pp = np.arange(P)
    kk = np.arange(KT)
    mT = ((kk[None, :, None] * P + pp[:, None, None]) // DV
          == hh[None, None, :]).astype(np.float32)
    shared["maskT"] = np.ascontiguousarray(
        np.concatenate([mT, np.zeros((P, KT, 1), np.float32)], axis=2))
    shared["U16"] = (hh[:, None] <= hh[None, :]).astype(np.float32)

    in_maps = []
    p64 = np.arange(64)
    for c in range(N_CORES):
        b, half = c // 2, c % 2
        s0 = half * S_LOC
        m = dict(shared)
        m["x_loc"] = np.ascontiguousarray(
            x[b, s0:s0 + S_LOC, :].astype(ml_dtypes.bfloat16))
        m["enc_loc"] = np.ascontiguousarray(
            enc[b, s0:s0 + S_LOC, :].astype(ml_dtypes.bfloat16))
        bsel = (p64[:, None] == 16 * b + hh[None, :]).astype(np.float32)
        m["Bsel"] = bsel
        m["BselT"] = np.ascontiguousarray(bsel.T)
        in_maps.append(m)
    return in_maps


def run_on_hw(inputs, **kwargs):
    nc = _get_nc(_affine_trivial(inputs))
    in_maps = _prep_inputs(inputs)
    return run_bass_kernel_spmd(nc, in_maps, list(range(N_CORES)), **kwargs)


def kernel(**inputs):
    r = run_on_hw(inputs)
    out = np.empty((B, S, D), dtype=np.float32)
    for c in range(N_CORES):
        b, half = c // 2, c % 2
        out[b, half * S_LOC:(half + 1) * S_LOC, :] = r.results[c]["out_loc"]
    return (out, np.zeros_like(out), np.zeros_like(out))


# revision 13
# speedup vs baseline: 1.3551x; 1.0129x over previous
"""Trainium2 Bass kernel for nn_DecoderLayer_43877385896448 (see spec).

Decoder layer with sigmoid linear attention (rank-1 per head), 2 attn blocks,
FFN, 3 layernorms.  B=4, S=4096, D=1024, H=16 heads (depth-1 q/k per head),
F=4096.

Sharding: rows (b, s) split across 8 cores -> core c owns batch b=c//2,
sequence half (c%2)*2048.  Row-parallel matmuls with replicated weights; the
only cross-core exchange is an AllReduce of the per-batch attention state
kv[16,65] per attention block, overlapped with compute.

Key algebra:
 - kv state: kv = sigk^T (X wv + bv) = ((sigk^T X) wv) + rowsum(sigk) bv, so
   the [S,D]x[D,D] v-projection collapses to a [16,D]x[D,D] after a cheap
   [16,S]x[S,D] accumulation that rides the row tiles (no v materialized).
 - attn out = [sigq ; 1] @ M_aug with M_aug = [blockdiag(cumsum kv) wo ; bo],
   eliminating the [S,D]x[D,D] output projection.
Weights on the fat paths are bf16 (q/k/v/o projections feed sigmoids / tiny
states; FFN measured 2e-3 rel err); residual adds ride the PE via identity
matmuls so PSUM drains fast and the PE HAM clock stays warm.
"""

import numpy as np
import ml_dtypes

import concourse.bass as bass
import concourse.bacc as bacc
import concourse.tile as tile
import concourse.mybir as mybir
from concourse import masks
from concourse.bass_utils import run_bass_kernel_spmd

F32 = mybir.dt.float32
F32R = mybir.dt.float32r
BF16 = mybir.dt.bfloat16
AF = mybir.ActivationFunctionType
ALU = mybir.AluOpType
AX = mybir.AxisListType

B, S, D, H, FF = 4, 4096, 1024, 16, 4096
DV = D // H            # 64
P = 128
N_CORES = 8
S_LOC = 2048           # rows per core
T = S_LOC // P         # 16 s-tiles per core
KT = D // P            # 8 k-tiles over D
MT = FF // P           # 32 dff tiles
EPS = 1e-6
SQ = 4                 # ffn processes s in 4 quarters of 512 rows
TQ = T // SQ
import os
NO_CC = bool(int(os.environ.get("BASS_NO_CC", "0")))


def build_program(affine_trivial=False):
    nc = bacc.Bacc("TRN2", target_bir_lowering=False, debug=False,
                   num_devices=N_CORES)

    d = {}

    def din(name, shape, dtype=F32):
        d[name] = nc.dram_tensor(name, list(shape), dtype,
                                 kind="ExternalInput").ap()

    din("x_loc", [S_LOC, D], BF16)
    din("enc_loc", [S_LOC, D], BF16)
    din("xTp", [T, P, KT, P], BF16)
    din("encTp", [T, P, KT, P], BF16)
    for w in ["wq1p", "wk1p", "wq2p", "wk2p"]:
        din(w, [P, KT, H], BF16)
    for w in ["wv1p", "wo1p", "wv2p", "wo2p"]:
        din(w, [P, KT, D], BF16)
    din("wf1p", [MT, P, KT, P], BF16)
    din("wf2p", [P, MT, D], BF16)
    din("bq1c", [H, 1]); din("bq2c", [H, 1])
    din("bk1b", [P, H]); din("bk2b", [P, H])
    din("bv1h", [H, DV]); din("bv2h", [H, DV])
    din("bo1r", [H + 1, D], BF16); din("bo2r", [H + 1, D], BF16)
    din("bf1c", [P, MT]); din("bf2b", [P, D], F32R)
    for v in ["g1b", "be1b", "g2b", "be2b", "g3b", "be3b"]:
        din(v, [P, D])
    din("maskh", [H, D]); din("maskT", [P, KT, H + 1])
    din("U16", [H, H]); din("Bsel", [64, H]); din("BselT", [H, 64])
    out_dram = nc.dram_tensor("out_loc", [S_LOC, D], F32,
                              kind="ExternalOutput").ap()

    with tile.TileContext(nc) as tc:
        def pool(name, bufs, side="left", space="SBUF"):
            return tc.alloc_tile_pool(name=name, bufs=bufs, side=side,
                                      space=space)

        # ---------------- constants ----------------
        cpool = pool("consts", 1)
        ident = cpool.tile([P, P], F32, name="ident")
        masks.make_identity(nc, ident[:])
        identr = cpool.tile([P, P], F32R, name="identr")
        nc.vector.tensor_copy(identr[:], ident[:])
        identb = cpool.tile([P, P], BF16, name="identb")
        nc.vector.tensor_copy(identb[:], ident[:])

        def load_const(key, dtype=F32, pl=None):
            pl = pl if pl is not None else cpool
            t_ = pl.tile([int(s) for s in d[key].shape], dtype,
                         name=f"c_{key}")
            nc.sync.dma_start(t_[:], d[key][:])
            return t_

        maskh = load_const("maskh")
        maskT = load_const("maskT")
        U16 = load_const("U16")
        Bsel = load_const("Bsel")
        BselT = load_const("BselT")
        bq1c = load_const("bq1c"); bq2c = load_const("bq2c")
        bk1b = load_const("bk1b"); bk2b = load_const("bk2b")
        bv1h = load_const("bv1h"); bv2h = load_const("bv2h")
        bo1r = load_const("bo1r", BF16); bo2r = load_const("bo2r", BF16)
        bf1c = load_const("bf1c")
        bf2br = load_const("bf2b", F32R)
        wq1 = load_const("wq1p", BF16); wk1 = load_const("wk1p", BF16)
        wq2 = load_const("wq2p", BF16); wk2 = load_const("wk2p", BF16)
        eps = cpool.tile([P, 1], F32, name="epsc")
        nc.vector.memset(eps[:], EPS)
        ones_col = cpool.tile([P, 1], F32, name="ones_col")
        nc.vector.memset(ones_col[:], 1.0)
        ones_colb = cpool.tile([P, 1], BF16, name="ones_colb")
        nc.vector.memset(ones_colb[:], 1.0)

        dram_pool = pool("ccdram", 1, space="DRAM")
        cc1_in = dram_pool.tile([64, 65], F32, name="cc1_in")
        cc1_out = dram_pool.tile([64, 65], F32, name="cc1_out")
        cc2_in = dram_pool.tile([64, 65], F32, name="cc2_in")
        cc2_out = dram_pool.tile([64, 65], F32, name="cc2_out")
        out1d = dram_pool.tile([S_LOC, D], BF16, name="out1d")

        # bigw: one 64KB/partition slot shared (in time) by wo1+wo2 during
        # phases A/B, then wf2 during the FFN (tag-shared, LIFO-friendly)
        bigw_pool = pool("bigw", 1)
        wo12 = bigw_pool.tile([P, 2 * KT, D], BF16, tag="w", name="wo12")
        for kt in range(KT):
            nc.sync.dma_start(wo12[:, kt, :], d["wo1p"][:, kt, :])
        for kt in range(KT):
            nc.gpsimd.dma_start(wo12[:, KT + kt, :], d["wo2p"][:, kt, :])

        # ---------------- long-lived left pools ----------------
        xrows_pool = pool("xrows", 1)
        xrows = xrows_pool.tile([P, T, D], BF16, name="xrows")
        xT_pool = pool("xT", 1)
        xT = xT_pool.tile([P, T, KT, P], BF16, name="xT")
        row_pool = pool("rows", 2)
        encT_pool = pool("encT", 2)
        wvs_pool = pool("wvs", 2)

        # ---------------- right pools (live whole program) ----------------
        sigk_pool = pool("sigk", 1, side="right")
        sigq1_pool = pool("sigq1", 1, side="right")
        sigq2_pool = pool("sigq2", 1, side="right")
        state_pool = pool("state", 1, side="right")
        ma1_pool = pool("ma1", 1, side="right")
        ma2_pool = pool("ma2", 1, side="right")
        ln_pool = pool("ln", 2, side="right")
        if affine_trivial:
            g1b = be1b = g2b = be2b = g3b = be3b = None
        else:
            gbe_pool = pool("gbe", 1, side="right")
            g1b = load_const("g1b", pl=gbe_pool)
            be1b = load_const("be1b", pl=gbe_pool)
            g2b = load_const("g2b", pl=gbe_pool)
            be2b = load_const("be2b", pl=gbe_pool)
            g3b = load_const("g3b", pl=gbe_pool)
            be3b = load_const("be3b", pl=gbe_pool)

        sigk1 = sigk_pool.tile([P, T, H], BF16, name="sigk1")
        sigk2 = sigk_pool.tile([P, T, H], BF16, name="sigk2")
        sigq1 = sigq1_pool.tile([H + 1, S_LOC], F32R, name="sigq1")
        sigq2 = sigq2_pool.tile([H + 1, S_LOC], F32R, name="sigq2")
        # row H stays 1.0 (the M_aug bias row); sigmoids overwrite rows 0:H
        nc.vector.memset(sigq1[:].bitcast(F32), 1.0)
        nc.vector.memset(sigq2[:].bitcast(F32), 1.0)

        # ---------------- helpers ----------------
        def transpose_into(src_ap, dst, col0, tp_pool, nm):
            """src [128, D] bf16 row tile -> dst[:, kt, col0:col0+128],
            transposed via normal matmuls (src stationary, identity moving:
            out = src^T @ I) — ~2x faster than transpose-mode and counts as
            PE activity for the HAM clock."""
            for half in range(2):
                tp = tp_pool.tile([P, 512], F32, tag="tp",
                                  name=f"tp_{nm}_{half}")
                for j in range(4):
                    kt = half * 4 + j
                    nc.tensor.matmul(tp[:, j * P:(j + 1) * P],
                                     src_ap[:, kt * P:(kt + 1) * P],
                                     identb[:], start=True, stop=True)
                dst_ap = dst[:, half * 4:(half + 1) * 4, col0:col0 + P]
                src_t = tp[:].rearrange("p (k n) -> p k n", k=4)
                if half == 0:
                    nc.vector.tensor_copy(dst_ap, src_t)
                else:
                    nc.scalar.copy(dst_ap, src_t)

        def ln_psum(chunks, outs, g_sb, be_sb, lnp, nm):
            """LayerNorm from psum chunks (total width D) -> outs dst APs.
            bn_stats is capped at 512 free elems; split wider chunks."""
            pieces = []
            for c in chunks:
                w = c.shape[-1]
                if w > 512:
                    for j in range(0, w, 512):
                        pieces.append(c[:, j:j + 512])
                else:
                    pieces.append(c)
            st6 = lnp.tile([P, len(pieces), 6], F32, tag="st6",
                           name=f"st6_{nm}")
            for i, c in enumerate(pieces):
                nc.vector.bn_stats(st6[:, i, :], c)
            mv = lnp.tile([P, 2], F32, tag="mv", name=f"mv_{nm}")
            nc.vector.bn_aggr(mv[:], st6[:])
            std = lnp.tile([P, 1], F32, tag="std", name=f"std_{nm}")
            nc.scalar.activation(std[:], mv[:, 1:2], AF.Sqrt, bias=eps[:])
            rstd = lnp.tile([P, 1], F32, tag="rstd", name=f"rstd_{nm}")
            nc.vector.reciprocal(rstd[:], std[:])
            nmr = lnp.tile([P, 1], F32, tag="nmr", name=f"nmr_{nm}")
            nc.vector.scalar_tensor_tensor(nmr[:], mv[:, 0:1], -1.0, rstd[:],
                                           op0=ALU.mult, op1=ALU.mult)
            off = 0
            for i, c in enumerate(chunks):
                w = c.shape[-1]
                if affine_trivial:
                    nc.scalar.activation(outs[i], c, AF.Identity,
                                         bias=nmr[:], scale=rstd[:])
                else:
                    sl = slice(off, off + w)
                    xh = lnp.tile([P, w], F32, tag="xh", name=f"xh_{nm}_{i}")
                    nc.scalar.activation(xh[:], c, AF.Identity,
                                         bias=nmr[:], scale=rstd[:])
                    nc.vector.scalar_tensor_tensor(xh[:], xh[:], 1.0,
                                                   g_sb[:, sl],
                                                   op0=ALU.mult, op1=ALU.mult)
                    nc.vector.tensor_add(outs[i], xh[:], be_sb[:, sl])
                off += w

        def kv_pack(A_ps, wv_key, sel_pool, cc_in, nm):
            """A_ps [16,1025] psum (sigk^T [X | 1]) -> G halves -> kv[16,65]
            -> batch-slot select [64,65] -> DMA for AllReduce."""
            asb = state_pool.tile([H, D + 1], F32, tag="asb",
                                  name=f"asb_{nm}")
            nc.vector.tensor_copy(asb[:], A_ps[:])
            atp = sel_pool.tile([P, KT * H], F32, tag="qc", name=f"atp_{nm}")
            for kt in range(KT):
                nc.tensor.matmul(atp[:, kt * H:(kt + 1) * H],
                                 asb[:, kt * P:(kt + 1) * P],
                                 ident[:H, :H], is_transpose=True)
            aT = state_pool.tile([P, KT, H], BF16, tag="aT", name=f"aT_{nm}")
            nc.vector.tensor_copy(aT[:],
                                  atp[:].rearrange("p (k h) -> p k h", k=KT))
            Ghs = [sel_pool.tile([H, 512], F32, tag="qc",
                                 name=f"G_{nm}_{half}") for half in range(2)]
            for kt in range(KT):
                wvc = wvs_pool.tile([P, D], BF16, tag="wv",
                                    name=f"wv_{nm}_{kt}")
                nc.scalar.dma_start(wvc[:], d[wv_key][:, kt, :])
                for half in range(2):
                    sl = slice(half * 512, (half + 1) * 512)
                    nc.tensor.matmul(Ghs[half][:], aT[:, kt, :], wvc[:, sl],
                                     start=(kt == 0), stop=(kt == KT - 1))
            kvph = []
            for half in range(2):
                sl = slice(half * 512, (half + 1) * 512)
                gt = state_pool.tile([H, 512], F32, tag="gt",
                                     name=f"gt_{nm}_{half}")
                nc.vector.tensor_mul(gt[:], Ghs[half][:], maskh[:, sl])
                kp = state_pool.tile([H, DV], F32, tag=f"kvp{half}",
                                     name=f"kvp_{nm}_{half}")
                nc.vector.tensor_reduce(
                    kp[:], gt[:].rearrange("p (c v) -> p v c", v=DV),
                    axis=AX.X, op=ALU.add)
                kvph.append(kp)
            kvp = state_pool.tile([H, DV + 1], F32, tag="kv",
                                  name=f"kv_{nm}")
            nc.vector.tensor_add(kvp[:, 0:DV], kvph[0][:], kvph[1][:])
            nc.vector.tensor_copy(kvp[:, DV:DV + 1], A_ps[:, D:D + 1])
            kvsel_ps = sel_pool.tile([64, 65], F32, tag="qc",
                                     name=f"kvselp_{nm}")
            nc.tensor.matmul(kvsel_ps[:], BselT[:], kvp[:],
                             start=True, stop=True)
            kvsel = state_pool.tile([64, 65], F32, tag="kvsel",
                                    name=f"kvsel_{nm}")
            nc.vector.tensor_copy(kvsel[:], kvsel_ps[:])
            nc.sync.dma_start(cc_in[:], kvsel[:])

        def state_to_M(cc_out, wo_off, bvh, bor, spsum, ma_pool, nm, ptag="qc"):
            """AllReduce out -> own-batch kv -> cumsum over heads ->
            M_aug [17,1024] (rows 0:16 blockdiag(cumsum kv) @ wo, row 16 bo)."""
            kvred = state_pool.tile([64, 65], F32, tag="kvred",
                                    name=f"kvred_{nm}")
            nc.sync.dma_start(kvred[:], cc_out[:])
            kvmy_ps = spsum.tile([H, 65], F32, tag=ptag, name=f"kvmyp_{nm}")
            nc.tensor.matmul(kvmy_ps[:], Bsel[:], kvred[:],
                             start=True, stop=True)
            kvmy = state_pool.tile([H, 65], F32, tag="kvmy",
                                   name=f"kvmy_{nm}")
            nc.vector.tensor_copy(kvmy[:], kvmy_ps[:])
            kv_bv = state_pool.tile([H, DV], F32, tag="kv_bv",
                                    name=f"kv_bv_{nm}")
            nc.vector.scalar_tensor_tensor(
                kv_bv[:], bvh[:], kvmy[:, DV:DV + 1], kvmy[:, 0:DV],
                op0=ALU.mult, op1=ALU.add)
            scum_ps = spsum.tile([H, DV], F32, tag=ptag, name=f"scump_{nm}")
            nc.tensor.matmul(scum_ps[:], U16[:], kv_bv[:],
                             start=True, stop=True)
            scum = state_pool.tile([H, DV], F32, tag="scumsb",
                                   name=f"scum_{nm}")
            nc.vector.tensor_copy(scum[:], scum_ps[:])
            scumT_ps = spsum.tile([DV, H], F32, tag=ptag, name=f"scumTp_{nm}")
            nc.tensor.matmul(scumT_ps[:], scum[:], ident[:H, :H],
                             is_transpose=True)
            scumT2 = state_pool.tile([P, H + 1], F32, tag="scumT2",
                                     name=f"scumT2_{nm}")
            nc.vector.memset(scumT2[:, H:H + 1], 0.0)
            nc.vector.tensor_copy(scumT2[0:DV, 0:H], scumT_ps[:])
            nc.vector.tensor_copy(scumT2[DV:P, 0:H], scumT_ps[:])
            bdts = []
            for kt in range(KT):
                bdt = state_pool.tile([P, H + 1], BF16, tag=f"bdt{kt}",
                                      name=f"bdt_{nm}_{kt}")
                nc.vector.tensor_mul(bdt[:], scumT2[:], maskT[:, kt, :])
                bdts.append(bdt)
            Ma = ma_pool.tile([H + 1, D], F32R, name=f"Ma_{nm}")
            for n2 in range(2):
                sl = slice(n2 * 512, (n2 + 1) * 512)
                M_ps = spsum.tile([H + 1, 512], F32, tag=ptag,
                                  name=f"M_{nm}_{n2}")
                nc.tensor.matmul(M_ps[:], identb[:H + 1, :H + 1], bor[:, sl],
                                 start=True, stop=False)
                for kt in range(KT):
                    nc.tensor.matmul(M_ps[:], bdts[kt][:],
                                     wo12[:, wo_off + kt, sl],
                                     start=False, stop=(kt == KT - 1))
                nc.vector.tensor_copy(Ma[:, sl], M_ps[:])
            return Ma

        # ================= PHASE A1: x side =================
        k_psum = pool("k_psum", 1, space="PSUM")
        A_psum = pool("A_psum", 1, space="PSUM")
        q1p = pool("q1p", 2, space="PSUM")

        A1_ps = A_psum.tile([H, D + 1], F32, tag="A", name="A1_ps")
        for t in range(T):
            nc.sync.dma_start(xT[:, t, :, :], d["xTp"][t])
            nc.gpsimd.dma_start(xrows[:, t, :],
                                d["x_loc"][t * P:(t + 1) * P, :])
            kp = k_psum.tile([P, H], F32, tag="kp", name=f"k1p_{t}")
            for kt in range(KT):
                nc.tensor.matmul(kp[:], xT[:, t, kt, :], wk1[:, kt, :],
                                 start=(kt == 0), stop=(kt == KT - 1))
            ktmp = state_pool.tile([P, H], F32, tag="ktmp", name=f"k1t_{t}")
            nc.vector.tensor_add(ktmp[:], kp[:], bk1b[:])
            nc.scalar.activation(sigk1[:, t, :], ktmp[:], AF.Sigmoid)
            for n2 in range(2):
                sl = slice(n2 * 512, (n2 + 1) * 512)
                nc.tensor.matmul(A1_ps[:, sl], sigk1[:, t, :],
                                 xrows[:, t, sl],
                                 start=(t == 0), stop=(t == T - 1))
            nc.tensor.matmul(A1_ps[:, D:D + 1],
                             sigk1[:, t, :], ones_colb[:],
                             start=(t == 0), stop=(t == T - 1))
            if t % 4 == 3:
                n = t // 4
                qp = q1p.tile([H, 512], F32, tag="qc", name=f"q1_{n}")
                for kt in range(KT):
                    nc.tensor.matmul(qp[:], wq1[:, kt, :],
                                     xT[:, 4 * n:4 * n + 4, kt, :],
                                     start=(kt == 0), stop=(kt == KT - 1))
                nc.scalar.activation(sigq1[0:H, n * 512:(n + 1) * 512],
                                     qp[:], AF.Sigmoid, bias=bq1c[:])

        kv_pack(A1_ps, "wv1p", q1p, cc1_in, "kv1")
        if NO_CC:
            nc.sync.dma_start(cc1_out[:], cc1_in[:])
        else:
            nc.gpsimd.collective_compute(
                "AllReduce", ALU.add, replica_groups=[list(range(N_CORES))],
                ins=[cc1_in.opt()], outs=[cc1_out.opt()])

        # ================= PHASE A2: enc side (overlaps AllReduce 1) ====
        A2_ps = A_psum.tile([H, D + 1], F32, tag="A", name="A2_ps")
        for t in range(T):
            er = row_pool.tile([P, D], BF16, tag="er", name=f"er_{t}")
            nc.gpsimd.dma_start(er[:], d["enc_loc"][t * P:(t + 1) * P, :])
            eT = encT_pool.tile([P, KT, P], BF16, tag="eT", name=f"eT_{t}")
            nc.scalar.dma_start(eT[:], d["encTp"][t])
            kp2 = k_psum.tile([P, H], F32, tag="kp", name=f"k2p_{t}")
            for kt in range(KT):
                nc.tensor.matmul(kp2[:], eT[:, kt, :], wk2[:, kt, :],
                                 start=(kt == 0), stop=(kt == KT - 1))
            ktmp2 = state_pool.tile([P, H], F32, tag="ktmp", name=f"k2t_{t}")
            nc.vector.tensor_add(ktmp2[:], kp2[:], bk2b[:])
            nc.scalar.activation(sigk2[:, t, :], ktmp2[:], AF.Sigmoid)
            for n2 in range(2):
                sl = slice(n2 * 512, (n2 + 1) * 512)
                nc.tensor.matmul(A2_ps[:, sl], sigk2[:, t, :], er[:, sl],
                                 start=(t == 0), stop=(t == T - 1))
            nc.tensor.matmul(A2_ps[:, D:D + 1],
                             sigk2[:, t, :], ones_colb[:],
                             start=(t == 0), stop=(t == T - 1))

        # m1 state chain: fills gaps while enc-side work runs; waits AR1
        Ma1 = state_to_M(cc1_out, 0, bv1h, bo1r, q1p, ma1_pool, "m1")

        kv_pack(A2_ps, "wv2p", q1p, cc2_in, "kv2")
        if NO_CC:
            nc.sync.dma_start(cc2_out[:], cc2_in[:])
        else:
            nc.gpsimd.collective_compute(
                "AllReduce", ALU.add, replica_groups=[list(range(N_CORES))],
                ins=[cc2_in.opt()], outs=[cc2_out.opt()])

        wvs_pool.release()
        encT_pool.release()
        row_pool.release()
        q1p.release()
        A_psum.release()
        k_psum.release()

        # ================= PHASE B: attn1 + LN1 + q2 =================
        ab_psum = pool("ab_psum", 2, space="PSUM")
        tpB = pool("tpB", 2, space="PSUM", side="right")
        q2p = pool("q2p", 2, space="PSUM", side="right")
        roll_pool = pool("roll", 2)

        roll = None
        for t in range(T):
            if t % 4 == 0:
                roll = roll_pool.tile([P, KT, 512], BF16, tag="roll",
                                      name=f"roll_{t // 4}")
            ap_ = ab_psum.tile([P, D], F32, tag="a", name=f"a1_{t}")
            for n2 in range(2):
                sl = slice(n2 * 512, (n2 + 1) * 512)
                nc.tensor.matmul(ap_[:, sl], sigq1[:, t * P:(t + 1) * P],
                                 Ma1[:, sl], start=True, stop=False)
                nc.tensor.matmul(ap_[:, sl], identb[:], xrows[:, t, sl],
                                 start=False, stop=True)
            o1t = ln_pool.tile([P, D], BF16, tag="ot", name=f"o1t_{t}")
            ln_psum([ap_[:]], [o1t[:]], g1b, be1b, ln_pool, f"ln1_{t}")
            eng = nc.sync if t % 2 == 0 else nc.gpsimd
            eng.dma_start(out1d[t * P:(t + 1) * P, :], o1t[:])
            transpose_into(o1t[:], roll, (t % 4) * P, tpB, f"o1{t}")
            if t % 4 == 3:
                n = t // 4
                qp2 = q2p.tile([H, 512], F32, tag="qc2", name=f"q2_{n}")
                for kt in range(KT):
                    nc.tensor.matmul(qp2[:], wq2[:, kt, :], roll[:, kt, :],
                                     start=(kt == 0), stop=(kt == KT - 1))
                nc.scalar.activation(sigq2[0:H, n * 512:(n + 1) * 512],
                                     qp2[:], AF.Sigmoid, bias=bq2c[:])

        # m2 chain rides the B tail (AR2 completed during B)
        Ma2 = state_to_M(cc2_out, KT, bv2h, bo2r, q2p, ma2_pool, "m2",
                         ptag="qc2")

        roll_pool.release()
        q2p.release()
        tpB.release()
        ab_psum.release()
        xT_pool.release()
        xrows_pool.release()

        # ================= PHASE C+D: attn2+LN2 + FFN =================
        a2_psum = pool("a2_psum", 3, space="PSUM", side="right")
        tpD = pool("tpD", 2, space="PSUM", side="right")
        h_psum = pool("h_psum", 3, space="PSUM")

        wf2t = bigw_pool.tile([P, MT, D], BF16, tag="w", name="wf2")
        for m in range(MT):
            nc.scalar.dma_start(wf2t[:, m, :], d["wf2p"][:, m, :])

        def wf2_ap(m, sl):
            return wf2t[:, m, sl]

        hT_pool = pool("hT", 1)
        o2T_pool = pool("o2T", 1)
        o2row_pool = pool("o2row", 2)
        wf1_pool = pool("wf1", 4)
        o1row_pool = pool("o1row", 2)
        o3_pool = pool("o3", 2)

        o2qs = [None] * SQ

        def attn2_group(g):
            o2q = o2row_pool.tile([P, TQ * D], BF16, tag="o2q",
                                  name=f"o2q_{g}")
            o2qs[g] = o2q
            for t4 in range(TQ):
                t = g * TQ + t4
                o1r = o1row_pool.tile([P, D], BF16, tag="o1r",
                                      name=f"o1r_{t}")
                nc.gpsimd.dma_start(o1r[:], out1d[t * P:(t + 1) * P, :])
                chunks, outs = [], []
                for n2 in range(2):
                    sl = slice(n2 * 512, (n2 + 1) * 512)
                    apc = a2_psum.tile([P, 512], F32, tag="a2",
                                       name=f"a2_{t}_{n2}")
                    nc.tensor.matmul(apc[:], sigq2[:, t * P:(t + 1) * P],
                                     Ma2[:, sl], start=True, stop=False)
                    nc.tensor.matmul(apc[:], identb[:], o1r[:, sl],
                                     start=False, stop=True)
                    chunks.append(apc[:])
                    outs.append(o2q[:, t4 * D + n2 * 512:t4 * D + (n2 + 1) * 512])
                ln_psum(chunks, outs, g2b, be2b, ln_pool, f"ln2_{t}")

        def ffn_block(g):
            o2q = o2qs[g]
            o2T = o2T_pool.tile([P, KT, TQ * P], BF16, tag="o2T",
                                name=f"o2T_{g}")
            for t4 in range(TQ):
                src = o2q[:, t4 * D:(t4 + 1) * D]
                transpose_into(src, o2T, t4 * P, tpD, f"o2{g}_{t4}")
            hT = hT_pool.tile([P, MT, TQ * P], BF16, tag="hT",
                              name=f"hT_{g}")
            for m in range(MT):
                wf1m = wf1_pool.tile([P, KT, P], BF16, tag="wf1m",
                                     name=f"wf1_{g}_{m}")
                eng = nc.gpsimd if m % 2 == 0 else nc.sync
                eng.dma_start(wf1m[:], d["wf1p"][m])
                hp = h_psum.tile([P, TQ * P], F32, tag="hp",
                                 name=f"hp_{g}_{m}")
                for kt in range(KT):
                    nc.tensor.matmul(hp[:], wf1m[:, kt, :], o2T[:, kt, :],
                                     start=(kt == 0), stop=(kt == KT - 1))
                nc.scalar.activation(hT[:, m, :], hp[:], AF.Relu,
                                     bias=bf1c[:, m:m + 1])
            for t4 in range(TQ):
                t = g * TQ + t4
                o3 = o3_pool.tile([P, D], F32, tag="o3", name=f"o3_{t}")
                chunks, outs = [], []
                for n2 in range(2):
                    sl = slice(n2 * 512, (n2 + 1) * 512)
                    op3 = h_psum.tile([P, 512], F32, tag="hp",
                                      name=f"o3c_{t}_{n2}")
                    for m in range(MT):
                        nc.tensor.matmul(op3[:],
                                         hT[:, m, t4 * P:(t4 + 1) * P],
                                         wf2_ap(m, sl),
                                         start=(m == 0), stop=False)
                    nc.tensor.matmul(op3[:], identb[:],
                                     o2q[:, t4 * D + n2 * 512:
                                          t4 * D + (n2 + 1) * 512],
                                     start=False, stop=affine_trivial)
                    if not affine_trivial:
                        nc.tensor.matmul(op3[:], identr[:], bf2br[:, sl],
                                         start=False, stop=True)
                    chunks.append(op3[:])
                    outs.append(o3[:, sl])
                ln_psum(chunks, outs, g3b, be3b, ln_pool, f"ln3_{t}")
                nc.sync.dma_start(out_dram[t * P:(t + 1) * P, :], o3[:])

        for g in range(SQ):
            attn2_group(g)
            if g >= 1:
                ffn_block(g - 1)
        ffn_block(SQ - 1)

        for p_ in [o3_pool, o1row_pool, wf1_pool, o2row_pool, o2T_pool,
                   hT_pool, h_psum, tpD, a2_psum,
                   bigw_pool, ln_pool, ma2_pool, ma1_pool,
                   state_pool, sigq2_pool, sigq1_pool, sigk_pool,
                   dram_pool, cpool]:
            p_.release()
        if not affine_trivial:
            gbe_pool.release()

    nc.compile()
    return nc


_NC_CACHE = {}


def _get_nc(affine_trivial):
    if affine_trivial not in _NC_CACHE:
        _NC_CACHE[affine_trivial] = build_program(affine_trivial)
    return _NC_CACHE[affine_trivial]


def _affine_trivial(inputs):
    for g in ("g1", "g2", "g3"):
        if not np.all(np.asarray(inputs[g]) == 1.0):
            return False
    for b in ("be1", "be2", "be3", "bf2"):
        if not np.all(np.asarray(inputs[b]) == 0.0):
            return False
    return True


def _prep_inputs(inputs):
    f32 = lambda a: np.ascontiguousarray(np.asarray(a, dtype=np.float32))
    bf = lambda a: np.ascontiguousarray(
        np.asarray(a, dtype=np.float32).astype(ml_dtypes.bfloat16))
    x = f32(inputs["x"])
    enc = f32(inputs["enc"])

    def pack_w(w):  # [D, n] -> [P, KT, n]
        w = f32(w)
        return np.ascontiguousarray(w.reshape(KT, P, -1).transpose(1, 0, 2))

    shared = {
        "wq1p": bf(pack_w(inputs["wq1"])), "wk1p": bf(pack_w(inputs["wk1"])),
        "wq2p": bf(pack_w(inputs["wq2"])), "wk2p": bf(pack_w(inputs["wk2"])),
        "wv1p": bf(pack_w(inputs["wv1"])), "wo1p": bf(pack_w(inputs["wo1"])),
        "wv2p": bf(pack_w(inputs["wv2"])), "wo2p": bf(pack_w(inputs["wo2"])),
    }
    wf1 = f32(inputs["wf1"])  # [D, FF]
    wf1p = wf1.reshape(KT, P, MT, P).transpose(2, 1, 0, 3)
    shared["wf1p"] = np.ascontiguousarray(wf1p.astype(ml_dtypes.bfloat16))
    wf2 = f32(inputs["wf2"])  # [FF, D]
    shared["wf2p"] = np.ascontiguousarray(
        wf2.reshape(MT, P, D).transpose(1, 0, 2).astype(ml_dtypes.bfloat16))

    def bcast(v):
        v = f32(v).reshape(-1)
        return np.ascontiguousarray(np.broadcast_to(v[None, :], (P, v.size)))

    def bo_aug(v):  # [17, D]: rows 0:16 zero, row 16 = bo
        m = np.zeros((H + 1, D), np.float32)
        m[H, :] = f32(v).reshape(-1)
        return np.ascontiguousarray(m.astype(ml_dtypes.bfloat16))

    shared["bq1c"] = f32(inputs["bq1"]).reshape(H, 1)
    shared["bq2c"] = f32(inputs["bq2"]).reshape(H, 1)
    shared["bk1b"] = bcast(inputs["bk1"])
    shared["bk2b"] = bcast(inputs["bk2"])
    shared["bv1h"] = f32(inputs["bv1"]).reshape(H, DV)
    shared["bv2h"] = f32(inputs["bv2"]).reshape(H, DV)
    shared["bo1r"] = bo_aug(inputs["bo1"])
    shared["bo2r"] = bo_aug(inputs["bo2"])
    shared["bf1c"] = np.ascontiguousarray(f32(inputs["bf1"]).reshape(MT, P).T)
    shared["bf2b"] = bcast(inputs["bf2"])
    for k_src, k_dst in [("g1", "g1b"), ("be1", "be1b"), ("g2", "g2b"),
                         ("be2", "be2b"), ("g3", "g3b"), ("be3", "be3b")]:
        shared[k_dst] = bcast(inputs[k_src])

    hh = np.arange(H)
    jj = np.arange(D)
    shared["maskh"] = (jj[None, :] // DV == hh[:, None]).astype(np.float32)
    pp = np.arange(P)
    kk = np.arange(KT)
    mT = ((kk[None, :, None] * P + pp[:, None, None]) // DV
          == hh[None, None, :]).astype(np.float32)
    shared["maskT"] = np.ascontiguousarray(
        np.concatenate([mT, np.zeros((P, KT, 1), np.float32)], axis=2))
    shared["U16"] = (hh[:, None] <= hh[None, :]).astype(np.float32)

    in_maps = []
    p64 = np.arange(64)
    for c in range(N_CORES):
        b, half = c // 2, c % 2
        s0 = half * S_LOC
        m = dict(shared)
        xl = x[b, s0:s0 + S_LOC, :].astype(ml_dtypes.bfloat16)
        el = enc[b, s0:s0 + S_LOC, :].astype(ml_dtypes.bfloat16)
        m["x_loc"] = np.ascontiguousarray(xl)
        m["enc_loc"] = np.ascontiguousarray(el)
        m["xTp"] = np.ascontiguousarray(
            xl.reshape(T, P, KT, P).transpose(0, 3, 2, 1))
        m["encTp"] = np.ascontiguousarray(
            el.reshape(T, P, KT, P).transpose(0, 3, 2, 1))
        bsel = (p64[:, None] == 16 * b + hh[None, :]).astype(np.float32)
        m["Bsel"] = bsel
        m["BselT"] = np.ascontiguousarray(bsel.T)
        in_maps.append(m)
    return in_maps


def run_on_hw(inputs, **kwargs):
    nc = _get_nc(_affine_trivial(inputs))
    in_maps = _prep_inputs(inputs)
    return run_bass_kernel_spmd(nc, in_maps, list(range(N_CORES)), **kwargs)


def kernel(**inputs):
    r = run_on_hw(inputs)
    out = np.empty((B, S, D), dtype=np.float32)
    for c in range(N_CORES):
        b, half = c // 2, c % 2
        out[b, half * S_LOC:(half + 1) * S_LOC, :] = r.results[c]["out_loc"]
    return (out, np.zeros_like(out), np.zeros_like(out))


# revision 14
# speedup vs baseline: 1.4010x; 1.0339x over previous
"""Trainium2 Bass kernel for nn_DecoderLayer_43877385896448 (see spec).

Decoder layer with sigmoid linear attention (rank-1 per head), 2 attn blocks,
FFN, 3 layernorms.  B=4, S=4096, D=1024, H=16 heads (depth-1 q/k per head),
F=4096.

Sharding: rows (b, s) split across 8 cores -> core c owns batch b=c//2,
sequence half (c%2)*2048.  Row-parallel matmuls with replicated weights; the
only cross-core exchange is an AllReduce of the per-batch attention state
kv[16,65] per attention block, overlapped with compute.

Key algebra:
 - kv state: kv = sigk^T (X wv + bv) = ((sigk^T X) wv) + rowsum(sigk) bv, so
   the [S,D]x[D,D] v-projection collapses to a [16,D]x[D,D] after a cheap
   [16,S]x[S,D] accumulation that rides the row tiles (no v materialized).
 - attn out = [sigq ; 1] @ M_aug with M_aug = [blockdiag(cumsum kv) wo ; bo],
   eliminating the [S,D]x[D,D] output projection.
Weights on the fat paths are bf16 (q/k/v/o projections feed sigmoids / tiny
states; FFN measured 2e-3 rel err); residual adds ride the PE via identity
matmuls so PSUM drains fast and the PE HAM clock stays warm.
"""

import numpy as np
import ml_dtypes

import concourse.bass as bass
import concourse.bacc as bacc
import concourse.tile as tile
import concourse.mybir as mybir
from concourse import masks
from concourse.bass_utils import run_bass_kernel_spmd

F32 = mybir.dt.float32
F32R = mybir.dt.float32r
BF16 = mybir.dt.bfloat16
AF = mybir.ActivationFunctionType
ALU = mybir.AluOpType
AX = mybir.AxisListType

B, S, D, H, FF = 4, 4096, 1024, 16, 4096
DV = D // H            # 64
P = 128
N_CORES = 8
S_LOC = 2048           # rows per core
T = S_LOC // P         # 16 s-tiles per core
KT = D // P            # 8 k-tiles over D
MT = FF // P           # 32 dff tiles
EPS = 1e-6
SQ = 4                 # ffn processes s in 4 quarters of 512 rows
TQ = T // SQ
import os
NO_CC = bool(int(os.environ.get("BASS_NO_CC", "0")))


def build_program(affine_trivial=False):
    nc = bacc.Bacc("TRN2", target_bir_lowering=False, debug=False,
                   num_devices=N_CORES)

    d = {}

    def din(name, shape, dtype=F32):
        d[name] = nc.dram_tensor(name, list(shape), dtype,
                                 kind="ExternalInput").ap()

    din("x_loc", [S_LOC, D], BF16)
    din("enc_loc", [S_LOC, D], BF16)
    din("xTp", [T, P, KT, P], BF16)
    din("encTp", [T, P, KT, P], BF16)
    for w in ["wq1p", "wk1p", "wq2p", "wk2p"]:
        din(w, [P, KT, H], BF16)
    for w in ["wv1p", "wo1p", "wv2p", "wo2p"]:
        din(w, [P, KT, D], BF16)
    din("wf1p", [MT, P, KT, P], BF16)
    din("wf2p", [P, MT, D], BF16)
    din("bq1c", [H, 1]); din("bq2c", [H, 1])
    din("bk1b", [P, H]); din("bk2b", [P, H])
    din("bv1h", [H, DV]); din("bv2h", [H, DV])
    din("bo1r", [H + 1, D], BF16); din("bo2r", [H + 1, D], BF16)
    din("bf1c", [P, MT]); din("bf2b", [P, D], F32R)
    for v in ["g1b", "be1b", "g2b", "be2b", "g3b", "be3b"]:
        din(v, [P, D])
    din("maskh", [H, D]); din("maskT", [P, KT, H + 1])
    din("U16", [H, H]); din("Bsel", [64, H]); din("BselT", [H, 64])
    out_dram = nc.dram_tensor("out_loc", [S_LOC, D], F32,
                              kind="ExternalOutput").ap()

    with tile.TileContext(nc) as tc:
        def pool(name, bufs, side="left", space="SBUF"):
            return tc.alloc_tile_pool(name=name, bufs=bufs, side=side,
                                      space=space)

        # ---------------- constants ----------------
        cpool = pool("consts", 1)
        ident = cpool.tile([P, P], F32, name="ident")
        masks.make_identity(nc, ident[:])
        identr = cpool.tile([P, P], F32R, name="identr")
        nc.vector.tensor_copy(identr[:], ident[:])
        identb = cpool.tile([P, P], BF16, name="identb")
        nc.vector.tensor_copy(identb[:], ident[:])

        def load_const(key, dtype=F32, pl=None):
            pl = pl if pl is not None else cpool
            t_ = pl.tile([int(s) for s in d[key].shape], dtype,
                         name=f"c_{key}")
            nc.sync.dma_start(t_[:], d[key][:])
            return t_

        maskh = load_const("maskh")
        maskT = load_const("maskT")
        U16 = load_const("U16")
        Bsel = load_const("Bsel")
        BselT = load_const("BselT")
        bq1c = load_const("bq1c"); bq2c = load_const("bq2c")
        bk1b = load_const("bk1b"); bk2b = load_const("bk2b")
        bv1h = load_const("bv1h"); bv2h = load_const("bv2h")
        bo1r = load_const("bo1r", BF16); bo2r = load_const("bo2r", BF16)
        bf1c = load_const("bf1c")
        bf2br = load_const("bf2b", F32R)
        wq1 = load_const("wq1p", BF16); wk1 = load_const("wk1p", BF16)
        wq2 = load_const("wq2p", BF16); wk2 = load_const("wk2p", BF16)
        eps = cpool.tile([P, 1], F32, name="epsc")
        nc.vector.memset(eps[:], EPS)
        ones_col = cpool.tile([P, 1], F32, name="ones_col")
        nc.vector.memset(ones_col[:], 1.0)
        ones_colb = cpool.tile([P, 1], BF16, name="ones_colb")
        nc.vector.memset(ones_colb[:], 1.0)

        dram_pool = pool("ccdram", 1, space="DRAM")
        cc1_in = dram_pool.tile([64, 65], F32, name="cc1_in")
        cc1_out = dram_pool.tile([64, 65], F32, name="cc1_out")
        cc2_in = dram_pool.tile([64, 65], F32, name="cc2_in")
        cc2_out = dram_pool.tile([64, 65], F32, name="cc2_out")
        out1d = dram_pool.tile([S_LOC, D], BF16, name="out1d")

        # bigw: one 64KB/partition slot shared (in time) by wo1+wo2 during
        # phases A/B, then wf2 during the FFN (tag-shared, LIFO-friendly)
        bigw_pool = pool("bigw", 1)
        wo12 = bigw_pool.tile([P, 2 * KT, D], BF16, tag="w", name="wo12")

        # ---------------- long-lived left pools ----------------
        xrows_pool = pool("xrows", 1)
        xrows = xrows_pool.tile([P, T, D], BF16, name="xrows")
        xT_pool = pool("xT", 1)
        xT = xT_pool.tile([P, T, KT, P], BF16, name="xT")
        row_pool = pool("rows", 4)
        encT_pool = pool("encT", 2)
        wvs_pool = pool("wvs", 2)

        # ---------------- right pools (live whole program) ----------------
        sigk_pool = pool("sigk", 1, side="right")
        sigq1_pool = pool("sigq1", 1, side="right")
        sigq2_pool = pool("sigq2", 1, side="right")
        state_pool = pool("state", 1, side="right")
        ma1_pool = pool("ma1", 1, side="right")
        ma2_pool = pool("ma2", 1, side="right")
        ln_pool = pool("ln", 2, side="right")
        if affine_trivial:
            g1b = be1b = g2b = be2b = g3b = be3b = None
        else:
            gbe_pool = pool("gbe", 1, side="right")
            g1b = load_const("g1b", pl=gbe_pool)
            be1b = load_const("be1b", pl=gbe_pool)
            g2b = load_const("g2b", pl=gbe_pool)
            be2b = load_const("be2b", pl=gbe_pool)
            g3b = load_const("g3b", pl=gbe_pool)
            be3b = load_const("be3b", pl=gbe_pool)

        sigk1 = sigk_pool.tile([P, T, H], BF16, name="sigk1")
        sigk2 = sigk_pool.tile([P, T, H], BF16, name="sigk2")
        sigq1 = sigq1_pool.tile([H + 1, S_LOC], F32R, name="sigq1")
        sigq2 = sigq2_pool.tile([H + 1, S_LOC], F32R, name="sigq2")
        # row H stays 1.0 (the M_aug bias row); sigmoids overwrite rows 0:H
        nc.vector.memset(sigq1[:].bitcast(F32), 1.0)
        nc.vector.memset(sigq2[:].bitcast(F32), 1.0)

        # ---------------- helpers ----------------
        def transpose_into(src_ap, dst, col0, tp_pool, nm):
            """src [128, D] bf16 row tile -> dst[:, kt, col0:col0+128],
            transposed via normal matmuls (src stationary, identity moving:
            out = src^T @ I) — ~2x faster than transpose-mode and counts as
            PE activity for the HAM clock."""
            for half in range(2):
                tp = tp_pool.tile([P, 512], F32, tag="tp",
                                  name=f"tp_{nm}_{half}")
                for j in range(4):
                    kt = half * 4 + j
                    nc.tensor.matmul(tp[:, j * P:(j + 1) * P],
                                     src_ap[:, kt * P:(kt + 1) * P],
                                     identb[:], start=True, stop=True)
                dst_ap = dst[:, half * 4:(half + 1) * 4, col0:col0 + P]
                src_t = tp[:].rearrange("p (k n) -> p k n", k=4)
                if half == 0:
                    nc.vector.tensor_copy(dst_ap, src_t)
                else:
                    nc.scalar.copy(dst_ap, src_t)

        def ln_psum(chunks, outs, g_sb, be_sb, lnp, nm):
            """LayerNorm from psum chunks (total width D) -> outs dst APs.
            bn_stats is capped at 512 free elems; split wider chunks."""
            pieces = []
            for c in chunks:
                w = c.shape[-1]
                if w > 512:
                    for j in range(0, w, 512):
                        pieces.append(c[:, j:j + 512])
                else:
                    pieces.append(c)
            st6 = lnp.tile([P, len(pieces), 6], F32, tag="st6",
                           name=f"st6_{nm}")
            for i, c in enumerate(pieces):
                nc.vector.bn_stats(st6[:, i, :], c)
            mv = lnp.tile([P, 2], F32, tag="mv", name=f"mv_{nm}")
            nc.vector.bn_aggr(mv[:], st6[:])
            std = lnp.tile([P, 1], F32, tag="std", name=f"std_{nm}")
            nc.scalar.activation(std[:], mv[:, 1:2], AF.Sqrt, bias=eps[:])
            rstd = lnp.tile([P, 1], F32, tag="rstd", name=f"rstd_{nm}")
            nc.vector.reciprocal(rstd[:], std[:])
            nmr = lnp.tile([P, 1], F32, tag="nmr", name=f"nmr_{nm}")
            nc.vector.scalar_tensor_tensor(nmr[:], mv[:, 0:1], -1.0, rstd[:],
                                           op0=ALU.mult, op1=ALU.mult)
            off = 0
            for i, c in enumerate(chunks):
                w = c.shape[-1]
                if affine_trivial:
                    nc.scalar.activation(outs[i], c, AF.Identity,
                                         bias=nmr[:], scale=rstd[:])
                else:
                    sl = slice(off, off + w)
                    xh = lnp.tile([P, w], F32, tag="xh", name=f"xh_{nm}_{i}")
                    nc.scalar.activation(xh[:], c, AF.Identity,
                                         bias=nmr[:], scale=rstd[:])
                    nc.vector.scalar_tensor_tensor(xh[:], xh[:], 1.0,
                                                   g_sb[:, sl],
                                                   op0=ALU.mult, op1=ALU.mult)
                    nc.vector.tensor_add(outs[i], xh[:], be_sb[:, sl])
                off += w

        def kv_pack(A_ps, wv_key, sel_pool, cc_in, nm):
            """A_ps [16,1025] psum (sigk^T [X | 1]) -> G halves -> kv[16,65]
            -> batch-slot select [64,65] -> DMA for AllReduce."""
            asb = state_pool.tile([H, D + 1], F32, tag="asb",
                                  name=f"asb_{nm}")
            nc.vector.tensor_copy(asb[:], A_ps[:])
            atp = sel_pool.tile([P, KT * H], F32, tag="qc", name=f"atp_{nm}")
            for kt in range(KT):
                nc.tensor.matmul(atp[:, kt * H:(kt + 1) * H],
                                 asb[:, kt * P:(kt + 1) * P],
                                 ident[:H, :H], is_transpose=True)
            aT = state_pool.tile([P, KT, H], BF16, tag="aT", name=f"aT_{nm}")
            nc.vector.tensor_copy(aT[:],
                                  atp[:].rearrange("p (k h) -> p k h", k=KT))
            Ghs = [sel_pool.tile([H, 512], F32, tag="qc",
                                 name=f"G_{nm}_{half}") for half in range(2)]
            for kt in range(KT):
                wvc = wvs_pool.tile([P, D], BF16, tag="wv",
                                    name=f"wv_{nm}_{kt}")
                nc.scalar.dma_start(wvc[:], d[wv_key][:, kt, :])
                for half in range(2):
                    sl = slice(half * 512, (half + 1) * 512)
                    nc.tensor.matmul(Ghs[half][:], aT[:, kt, :], wvc[:, sl],
                                     start=(kt == 0), stop=(kt == KT - 1))
            kvph = []
            for half in range(2):
                sl = slice(half * 512, (half + 1) * 512)
                gt = state_pool.tile([H, 512], F32, tag="gt",
                                     name=f"gt_{nm}_{half}")
                nc.vector.tensor_mul(gt[:], Ghs[half][:], maskh[:, sl])
                kp = state_pool.tile([H, DV], F32, tag=f"kvp{half}",
                                     name=f"kvp_{nm}_{half}")
                nc.vector.tensor_reduce(
                    kp[:], gt[:].rearrange("p (c v) -> p v c", v=DV),
                    axis=AX.X, op=ALU.add)
                kvph.append(kp)
            kvp = state_pool.tile([H, DV + 1], F32, tag="kv",
                                  name=f"kv_{nm}")
            nc.vector.tensor_add(kvp[:, 0:DV], kvph[0][:], kvph[1][:])
            nc.vector.tensor_copy(kvp[:, DV:DV + 1], A_ps[:, D:D + 1])
            kvsel_ps = sel_pool.tile([64, 65], F32, tag="qc",
                                     name=f"kvselp_{nm}")
            nc.tensor.matmul(kvsel_ps[:], BselT[:], kvp[:],
                             start=True, stop=True)
            kvsel = state_pool.tile([64, 65], F32, tag="kvsel",
                                    name=f"kvsel_{nm}")
            nc.vector.tensor_copy(kvsel[:], kvsel_ps[:])
            nc.sync.dma_start(cc_in[:], kvsel[:])

        def state_to_M(cc_out, wo_off, bvh, bor, spsum, ma_pool, nm, ptag="qc"):
            """AllReduce out -> own-batch kv -> cumsum over heads ->
            M_aug [17,1024] (rows 0:16 blockdiag(cumsum kv) @ wo, row 16 bo)."""
            kvred = state_pool.tile([64, 65], F32, tag="kvred",
                                    name=f"kvred_{nm}")
            nc.sync.dma_start(kvred[:], cc_out[:])
            kvmy_ps = spsum.tile([H, 65], F32, tag=ptag, name=f"kvmyp_{nm}")
            nc.tensor.matmul(kvmy_ps[:], Bsel[:], kvred[:],
                             start=True, stop=True)
            kvmy = state_pool.tile([H, 65], F32, tag="kvmy",
                                   name=f"kvmy_{nm}")
            nc.vector.tensor_copy(kvmy[:], kvmy_ps[:])
            kv_bv = state_pool.tile([H, DV], F32, tag="kv_bv",
                                    name=f"kv_bv_{nm}")
            nc.vector.scalar_tensor_tensor(
                kv_bv[:], bvh[:], kvmy[:, DV:DV + 1], kvmy[:, 0:DV],
                op0=ALU.mult, op1=ALU.add)
            scum_ps = spsum.tile([H, DV], F32, tag=ptag, name=f"scump_{nm}")
            nc.tensor.matmul(scum_ps[:], U16[:], kv_bv[:],
                             start=True, stop=True)
            scum = state_pool.tile([H, DV], F32, tag="scumsb",
                                   name=f"scum_{nm}")
            nc.vector.tensor_copy(scum[:], scum_ps[:])
            scumT_ps = spsum.tile([DV, H], F32, tag=ptag, name=f"scumTp_{nm}")
            nc.tensor.matmul(scumT_ps[:], scum[:], ident[:H, :H],
                             is_transpose=True)
            scumT2 = state_pool.tile([P, H + 1], F32, tag="scumT2",
                                     name=f"scumT2_{nm}")
            nc.vector.memset(scumT2[:, H:H + 1], 0.0)
            nc.vector.tensor_copy(scumT2[0:DV, 0:H], scumT_ps[:])
            nc.vector.tensor_copy(scumT2[DV:P, 0:H], scumT_ps[:])
            bdts = []
            for kt in range(KT):
                bdt = state_pool.tile([P, H + 1], BF16, tag=f"bdt{kt}",
                                      name=f"bdt_{nm}_{kt}")
                nc.vector.tensor_mul(bdt[:], scumT2[:], maskT[:, kt, :])
                bdts.append(bdt)
            Ma = ma_pool.tile([H + 1, D], F32R, name=f"Ma_{nm}")
            for n2 in range(2):
                sl = slice(n2 * 512, (n2 + 1) * 512)
                M_ps = spsum.tile([H + 1, 512], F32, tag=ptag,
                                  name=f"M_{nm}_{n2}")
                nc.tensor.matmul(M_ps[:], identb[:H + 1, :H + 1], bor[:, sl],
                                 start=True, stop=False)
                for kt in range(KT):
                    nc.tensor.matmul(M_ps[:], bdts[kt][:],
                                     wo12[:, wo_off + kt, sl],
                                     start=False, stop=(kt == KT - 1))
                nc.vector.tensor_copy(Ma[:, sl], M_ps[:])
            return Ma

        # ================= PHASE A1: x side =================
        k_psum = pool("k_psum", 3, space="PSUM")
        A_psum = pool("A_psum", 1, space="PSUM")
        q1p = pool("q1p", 2, space="PSUM")

        A1_ps = A_psum.tile([H, D + 1], F32, tag="A", name="A1_ps")
        for t in range(T):
            nc.sync.dma_start(xT[:, t, :, :], d["xTp"][t])
            nc.gpsimd.dma_start(xrows[:, t, :],
                                d["x_loc"][t * P:(t + 1) * P, :])
            kp = k_psum.tile([P, H], F32, tag="kp", name=f"k1p_{t}")
            for kt in range(KT):
                nc.tensor.matmul(kp[:], xT[:, t, kt, :], wk1[:, kt, :],
                                 start=(kt == 0), stop=(kt == KT - 1))
            ktmp = state_pool.tile([P, H], F32, tag="ktmp", name=f"k1t_{t}")
            nc.vector.tensor_add(ktmp[:], kp[:], bk1b[:])
            nc.scalar.activation(sigk1[:, t, :], ktmp[:], AF.Sigmoid)
            for n2 in range(2):
                sl = slice(n2 * 512, (n2 + 1) * 512)
                nc.tensor.matmul(A1_ps[:, sl], sigk1[:, t, :],
                                 xrows[:, t, sl],
                                 start=(t == 0), stop=(t == T - 1))
            nc.tensor.matmul(A1_ps[:, D:D + 1],
                             sigk1[:, t, :], ones_colb[:],
                             start=(t == 0), stop=(t == T - 1))
            if t % 4 == 3:
                n = t // 4
                qp = q1p.tile([H, 512], F32, tag="qc", name=f"q1_{n}")
                for kt in range(KT):
                    nc.tensor.matmul(qp[:], wq1[:, kt, :],
                                     xT[:, 4 * n:4 * n + 4, kt, :],
                                     start=(kt == 0), stop=(kt == KT - 1))
                nc.scalar.activation(sigq1[0:H, n * 512:(n + 1) * 512],
                                     qp[:], AF.Sigmoid, bias=bq1c[:])

        kv_pack(A1_ps, "wv1p", q1p, cc1_in, "kv1")
        if NO_CC:
            nc.sync.dma_start(cc1_out[:], cc1_in[:])
        else:
            nc.gpsimd.collective_compute(
                "AllReduce", ALU.add, replica_groups=[list(range(N_CORES))],
                ins=[cc1_in.opt()], outs=[cc1_out.opt()])

        # ================= PHASE A2: enc side (overlaps AllReduce 1) ====
        A2_ps = A_psum.tile([H, D + 1], F32, tag="A", name="A2_ps")
        for t in range(T):
            er = row_pool.tile([P, D], BF16, tag="er", name=f"er_{t}")
            nc.sync.dma_start(er[:], d["enc_loc"][t * P:(t + 1) * P, :])
            eT = encT_pool.tile([P, KT, P], BF16, tag="eT", name=f"eT_{t}")
            nc.scalar.dma_start(eT[:], d["encTp"][t])
            kp2 = k_psum.tile([P, H], F32, tag="kp", name=f"k2p_{t}")
            for kt in range(KT):
                nc.tensor.matmul(kp2[:], eT[:, kt, :], wk2[:, kt, :],
                                 start=(kt == 0), stop=(kt == KT - 1))
            ktmp2 = state_pool.tile([P, H], F32, tag="ktmp", name=f"k2t_{t}")
            nc.vector.tensor_add(ktmp2[:], kp2[:], bk2b[:])
            nc.scalar.activation(sigk2[:, t, :], ktmp2[:], AF.Sigmoid)
            for n2 in range(2):
                sl = slice(n2 * 512, (n2 + 1) * 512)
                nc.tensor.matmul(A2_ps[:, sl], sigk2[:, t, :], er[:, sl],
                                 start=(t == 0), stop=(t == T - 1))
            nc.tensor.matmul(A2_ps[:, D:D + 1],
                             sigk2[:, t, :], ones_colb[:],
                             start=(t == 0), stop=(t == T - 1))

        for kt in range(KT):
            nc.sync.dma_start(wo12[:, kt, :], d["wo1p"][:, kt, :])
        for kt in range(KT):
            nc.sync.dma_start(wo12[:, KT + kt, :], d["wo2p"][:, kt, :])

        # m1 state chain: fills gaps while enc-side work runs; waits AR1
        Ma1 = state_to_M(cc1_out, 0, bv1h, bo1r, q1p, ma1_pool, "m1")

        kv_pack(A2_ps, "wv2p", q1p, cc2_in, "kv2")
        if NO_CC:
            nc.sync.dma_start(cc2_out[:], cc2_in[:])
        else:
            nc.gpsimd.collective_compute(
                "AllReduce", ALU.add, replica_groups=[list(range(N_CORES))],
                ins=[cc2_in.opt()], outs=[cc2_out.opt()])

        wvs_pool.release()
        encT_pool.release()
        row_pool.release()
        q1p.release()
        A_psum.release()
        k_psum.release()

        # ================= PHASE B: attn1 + LN1 + q2 =================
        ab_psum = pool("ab_psum", 2, space="PSUM")
        tpB = pool("tpB", 2, space="PSUM", side="right")
        q2p = pool("q2p", 2, space="PSUM", side="right")
        roll_pool = pool("roll", 2)

        roll = None
        for t in range(T):
            if t % 4 == 0:
                roll = roll_pool.tile([P, KT, 512], BF16, tag="roll",
                                      name=f"roll_{t // 4}")
            ap_ = ab_psum.tile([P, D], F32, tag="a", name=f"a1_{t}")
            for n2 in range(2):
                sl = slice(n2 * 512, (n2 + 1) * 512)
                nc.tensor.matmul(ap_[:, sl], sigq1[:, t * P:(t + 1) * P],
                                 Ma1[:, sl], start=True, stop=False)
                nc.tensor.matmul(ap_[:, sl], identb[:], xrows[:, t, sl],
                                 start=False, stop=True)
            o1t = ln_pool.tile([P, D], BF16, tag="ot", name=f"o1t_{t}")
            ln_psum([ap_[:]], [o1t[:]], g1b, be1b, ln_pool, f"ln1_{t}")
            nc.sync.dma_start(out1d[t * P:(t + 1) * P, :], o1t[:])
            transpose_into(o1t[:], roll, (t % 4) * P, tpB, f"o1{t}")
            if t % 4 == 3:
                n = t // 4
                qp2 = q2p.tile([H, 512], F32, tag="qc2", name=f"q2_{n}")
                for kt in range(KT):
                    nc.tensor.matmul(qp2[:], wq2[:, kt, :], roll[:, kt, :],
                                     start=(kt == 0), stop=(kt == KT - 1))
                nc.scalar.activation(sigq2[0:H, n * 512:(n + 1) * 512],
                                     qp2[:], AF.Sigmoid, bias=bq2c[:])

        # m2 chain rides the B tail (AR2 completed during B)
        Ma2 = state_to_M(cc2_out, KT, bv2h, bo2r, q2p, ma2_pool, "m2",
                         ptag="qc2")

        roll_pool.release()
        q2p.release()
        tpB.release()
        ab_psum.release()
        xT_pool.release()
        xrows_pool.release()

        # ================= PHASE C+D: attn2+LN2 + FFN =================
        a2_psum = pool("a2_psum", 3, space="PSUM", side="right")
        tpD = pool("tpD", 2, space="PSUM", side="right")
        h_psum = pool("h_psum", 3, space="PSUM")

        wf2t = bigw_pool.tile([P, MT, D], BF16, tag="w", name="wf2")
        for m in range(MT):
            nc.scalar.dma_start(wf2t[:, m, :], d["wf2p"][:, m, :])

        def wf2_ap(m, sl):
            return wf2t[:, m, sl]

        hT_pool = pool("hT", 1)
        o2T_pool = pool("o2T", 1)
        o2row_pool = pool("o2row", 2)
        wf1_pool = pool("wf1", 4)
        o1row_pool = pool("o1row", 2)
        o3_pool = pool("o3", 2)

        o2qs = [None] * SQ

        def attn2_group(g):
            o2q = o2row_pool.tile([P, TQ * D], BF16, tag="o2q",
                                  name=f"o2q_{g}")
            o2qs[g] = o2q
            for t4 in range(TQ):
                t = g * TQ + t4
                o1r = o1row_pool.tile([P, D], BF16, tag="o1r",
                                      name=f"o1r_{t}")
                nc.sync.dma_start(o1r[:], out1d[t * P:(t + 1) * P, :])
                chunks, outs = [], []
                for n2 in range(2):
                    sl = slice(n2 * 512, (n2 + 1) * 512)
                    apc = a2_psum.tile([P, 512], F32, tag="a2",
                                       name=f"a2_{t}_{n2}")
                    nc.tensor.matmul(apc[:], sigq2[:, t * P:(t + 1) * P],
                                     Ma2[:, sl], start=True, stop=False)
                    nc.tensor.matmul(apc[:], identb[:], o1r[:, sl],
                                     start=False, stop=True)
                    chunks.append(apc[:])
                    outs.append(o2q[:, t4 * D + n2 * 512:t4 * D + (n2 + 1) * 512])
                ln_psum(chunks, outs, g2b, be2b, ln_pool, f"ln2_{t}")

        def ffn_block(g):
            o2q = o2qs[g]
            o2T = o2T_pool.tile([P, KT, TQ * P], BF16, tag="o2T",
                                name=f"o2T_{g}")
            for t4 in range(TQ):
                src = o2q[:, t4 * D:(t4 + 1) * D]
                transpose_into(src, o2T, t4 * P, tpD, f"o2{g}_{t4}")
            hT = hT_pool.tile([P, MT, TQ * P], BF16, tag="hT",
                              name=f"hT_{g}")
            for m in range(MT):
                wf1m = wf1_pool.tile([P, KT, P], BF16, tag="wf1m",
                                     name=f"wf1_{g}_{m}")
                eng = nc.gpsimd if m % 2 == 0 else nc.sync
                eng.dma_start(wf1m[:], d["wf1p"][m])
                hp = h_psum.tile([P, TQ * P], F32, tag="hp",
                                 name=f"hp_{g}_{m}")
                for kt in range(KT):
                    nc.tensor.matmul(hp[:], wf1m[:, kt, :], o2T[:, kt, :],
                                     start=(kt == 0), stop=(kt == KT - 1))
                nc.scalar.activation(hT[:, m, :], hp[:], AF.Relu,
                                     bias=bf1c[:, m:m + 1])
            for t4 in range(TQ):
                t = g * TQ + t4
                o3 = o3_pool.tile([P, D], F32, tag="o3", name=f"o3_{t}")
                chunks, outs = [], []
                for n2 in range(2):
                    sl = slice(n2 * 512, (n2 + 1) * 512)
                    op3 = h_psum.tile([P, 512], F32, tag="hp",
                                      name=f"o3c_{t}_{n2}")
                    for m in range(MT):
                        nc.tensor.matmul(op3[:],
                                         hT[:, m, t4 * P:(t4 + 1) * P],
                                         wf2_ap(m, sl),
                                         start=(m == 0), stop=False)
                    nc.tensor.matmul(op3[:], identb[:],
                                     o2q[:, t4 * D + n2 * 512:
                                          t4 * D + (n2 + 1) * 512],
                                     start=False, stop=affine_trivial)
                    if not affine_trivial:
                        nc.tensor.matmul(op3[:], identr[:], bf2br[:, sl],
                                         start=False, stop=True)
                    chunks.append(op3[:])
                    outs.append(o3[:, sl])
                ln_psum(chunks, outs, g3b, be3b, ln_pool, f"ln3_{t}")
                nc.sync.dma_start(out_dram[t * P:(t + 1) * P, :], o3[:])

        for g in range(SQ):
            attn2_group(g)
            if g >= 1:
                ffn_block(g - 1)
        ffn_block(SQ - 1)

        for p_ in [o3_pool, o1row_pool, wf1_pool, o2row_pool, o2T_pool,
                   hT_pool, h_psum, tpD, a2_psum,
                   bigw_pool, ln_pool, ma2_pool, ma1_pool,
                   state_pool, sigq2_pool, sigq1_pool, sigk_pool,
                   dram_pool, cpool]:
            p_.release()
        if not affine_trivial:
            gbe_pool.release()

    nc.compile()
    return nc


_NC_CACHE = {}


def _get_nc(affine_trivial):
    if affine_trivial not in _NC_CACHE:
        _NC_CACHE[affine_trivial] = build_program(affine_trivial)
    return _NC_CACHE[affine_trivial]


def _affine_trivial(inputs):
    for g in ("g1", "g2", "g3"):
        if not np.all(np.asarray(inputs[g]) == 1.0):
            return False
    for b in ("be1", "be2", "be3", "bf2"):
        if not np.all(np.asarray(inputs[b]) == 0.0):
            return False
    return True


def _prep_inputs(inputs):
    f32 = lambda a: np.ascontiguousarray(np.asarray(a, dtype=np.float32))
    bf = lambda a: np.ascontiguousarray(
        np.asarray(a, dtype=np.float32).astype(ml_dtypes.bfloat16))
    x = f32(inputs["x"])
    enc = f32(inputs["enc"])

    def pack_w(w):  # [D, n] -> [P, KT, n]
        w = f32(w)
        return np.ascontiguousarray(w.reshape(KT, P, -1).transpose(1, 0, 2))

    shared = {
        "wq1p": bf(pack_w(inputs["wq1"])), "wk1p": bf(pack_w(inputs["wk1"])),
        "wq2p": bf(pack_w(inputs["wq2"])), "wk2p": bf(pack_w(inputs["wk2"])),
        "wv1p": bf(pack_w(inputs["wv1"])), "wo1p": bf(pack_w(inputs["wo1"])),
        "wv2p": bf(pack_w(inputs["wv2"])), "wo2p": bf(pack_w(inputs["wo2"])),
    }
    wf1 = f32(inputs["wf1"])  # [D, FF]
    wf1p = wf1.reshape(KT, P, MT, P).transpose(2, 1, 0, 3)
    shared["wf1p"] = np.ascontiguousarray(wf1p.astype(ml_dtypes.bfloat16))
    wf2 = f32(inputs["wf2"])  # [FF, D]
    shared["wf2p"] = np.ascontiguousarray(
        wf2.reshape(MT, P, D).transpose(1, 0, 2).astype(ml_dtypes.bfloat16))

    def bcast(v):
        v = f32(v).reshape(-1)
        return np.ascontiguousarray(np.broadcast_to(v[None, :], (P, v.size)))

    def bo_aug(v):  # [17, D]: rows 0:16 zero, row 16 = bo
        m = np.zeros((H + 1, D), np.float32)
        m[H, :] = f32(v).reshape(-1)
        return np.ascontiguousarray(m.astype(ml_dtypes.bfloat16))

    shared["bq1c"] = f32(inputs["bq1"]).reshape(H, 1)
    shared["bq2c"] = f32(inputs["bq2"]).reshape(H, 1)
    shared["bk1b"] = bcast(inputs["bk1"])
    shared["bk2b"] = bcast(inputs["bk2"])
    shared["bv1h"] = f32(inputs["bv1"]).reshape(H, DV)
    shared["bv2h"] = f32(inputs["bv2"]).reshape(H, DV)
    shared["bo1r"] = bo_aug(inputs["bo1"])
    shared["bo2r"] = bo_aug(inputs["bo2"])
    shared["bf1c"] = np.ascontiguousarray(f32(inputs["bf1"]).reshape(MT, P).T)
    shared["bf2b"] = bcast(inputs["bf2"])
    for k_src, k_dst in [("g1", "g1b"), ("be1", "be1b"), ("g2", "g2b"),
                         ("be2", "be2b"), ("g3", "g3b"), ("be3", "be3b")]:
        shared[k_dst] = bcast(inputs[k_src])

    hh = np.arange(H)
    jj = np.arange(D)
    shared["maskh"] = (jj[None, :] // DV == hh[:, None]).astype(np.float32)
    pp = np.arange(P)
    kk = np.arange(KT)
    mT = ((kk[None, :, None] * P + pp[:, None, None]) // DV
          == hh[None, None, :]).astype(np.float32)
    shared["maskT"] = np.ascontiguousarray(
        np.concatenate([mT, np.zeros((P, KT, 1), np.float32)], axis=2))
    shared["U16"] = (hh[:, None] <= hh[None, :]).astype(np.float32)

    in_maps = []
    p64 = np.arange(64)
    for c in range(N_CORES):
        b, half = c // 2, c % 2
        s0 = half * S_LOC
        m = dict(shared)
        xl = x[b, s0:s0 + S_LOC, :].astype(ml_dtypes.bfloat16)
        el = enc[b, s0:s0 + S_LOC, :].astype(ml_dtypes.bfloat16)
        m["x_loc"] = np.ascontiguousarray(xl)
        m["enc_loc"] = np.ascontiguousarray(el)
        m["xTp"] = np.ascontiguousarray(
            xl.reshape(T, P, KT, P).transpose(0, 3, 2, 1))
        m["encTp"] = np.ascontiguousarray(
            el.reshape(T, P, KT, P).transpose(0, 3, 2, 1))
        bsel = (p64[:, None] == 16 * b + hh[None, :]).astype(np.float32)
        m["Bsel"] = bsel
        m["BselT"] = np.ascontiguousarray(bsel.T)
        in_maps.append(m)
    return in_maps


def run_on_hw(inputs, **kwargs):
    nc = _get_nc(_affine_trivial(inputs))
    in_maps = _prep_inputs(inputs)
    return run_bass_kernel_spmd(nc, in_maps, list(range(N_CORES)), **kwargs)


def kernel(**inputs):
    r = run_on_hw(inputs)
    out = np.empty((B, S, D), dtype=np.float32)
    for c in range(N_CORES):
        b, half = c // 2, c % 2
        out[b, half * S_LOC:(half + 1) * S_LOC, :] = r.results[c]["out_loc"]
    return (out, np.zeros_like(out), np.zeros_like(out))
